# revision 1
# baseline (speedup 1.0000x reference)
"""Trainium2 Bass kernel for the QRNN-style recommender model.

Model (per batch row b):
  emb = item_emb[seq]                          # [T=16, D=256]
  conv_out[l,t,c] = sum_{m<=l} emb[t-m] @ W[l,m,c,:] + conv_b[l,c]   (L=16 causal convs)
  f = sigmoid(relu(conv_out))                  # forget gates
  h = fo-pool chain applied 3x over t (QRNN), x0 = emb
  o = sum over (l, t) of h                     # [D]
  z = [o, user_emb[user]] @ fc1_w.T + fc1_b    # [D]
  res[n] = W2[item[n]] . z + b2[item[n]]       # [N_TGT=32]

Sharding: data-parallel over batch B=512 across 8 cores (64 rows each);
all parameters/tables replicated; embedding gathers run on-device via
indirect DMA.

Per-core device layout:
  embT[kc][d(128), b(64), tpad(31)]  time-padded transposed gathered emb
  conv: psum[c(128), (b,t)(1024)] accumulated over (m, kc) with shifted
        time windows of embT; f32r matmuls (full-rate for N=512)
  gates: ACT relu(z+b) -> r; f = sigmoid(r); g = sigmoid(-r) = 1-f
  fo-pool: DVE tensor_tensor_scan (state = g*state + f*x) over a
        (b, 17)-slotted free dim; slot 0 per b is a reset (g=f*x=0)
  head: fc1 via PE, per-row dot with gathered W2 rows via DVE mul +
        ones-vector PE partition-reduction.
"""
import os
import numpy as np

import concourse.bass as bass
import concourse.mybir as mybir
import concourse.tile as tile
from concourse import bacc
from concourse.masks import make_identity

F32 = mybir.dt.float32
F32R = mybir.dt.float32r
BF16 = mybir.dt.bfloat16
I32 = mybir.dt.int32
AF = mybir.ActivationFunctionType
ALU = mybir.AluOpType

# model dims (hardcoded per problem spec)
N_CORES = 8
B = 512
BC = B // N_CORES          # 64 rows per core
T = 16
L = 16
D = 256
N_TGT = 32
N_ITEMS = 200000
N_USERS = 100000
N_L = 3                    # fo-pool chain depth
PAD = L - 1                # 15 zero columns of left time padding
TW = T + PAD               # 31
S = T + 1                  # 17 scan slots per b (slot 0 = reset)
TRI = [l * (l + 1) // 2 for l in range(L + 1)]  # block offsets for (l, m<=l)


def _build_kernel(nc, tc):
    seq8 = nc.dram_tensor("seq8", [8, 128], I32, kind="ExternalInput").ap()
    item16 = nc.dram_tensor("item16", [16, 128], I32, kind="ExternalInput").ap()
    useri = nc.dram_tensor("useri", [BC], I32, kind="ExternalInput").ap()
    item_emb = nc.dram_tensor("item_emb", [N_ITEMS, D], F32, kind="ExternalInput").ap()
    user_emb = nc.dram_tensor("user_emb", [N_USERS, D], F32, kind="ExternalInput").ap()
    w2tab = nc.dram_tensor("w2tab", [N_ITEMS, D], F32, kind="ExternalInput").ap()
    wt = nc.dram_tensor("wt", [TRI[L], D, D], BF16, kind="ExternalInput").ap()
    convb = nc.dram_tensor("convb", [128, 2, L], F32, kind="ExternalInput").ap()
    fc1wt = nc.dram_tensor("fc1wt", [2 * D, D], F32, kind="ExternalInput").ap()
    fc1b = nc.dram_tensor("fc1b", [128, 2], F32, kind="ExternalInput").ap()
    res = nc.dram_tensor("res", [BC, N_TGT], F32, kind="ExternalOutput").ap()

    import contextlib
    ctx = contextlib.ExitStack()
    with ctx:
        perm = ctx.enter_context(tc.tile_pool(name="perm", bufs=1))
        idxp = ctx.enter_context(tc.tile_pool(name="idxp", bufs=2))
        gath = ctx.enter_context(tc.tile_pool(name="gath", bufs=4))
        wpool = ctx.enter_context(tc.tile_pool(name="wpool", bufs=8))
        rp = ctx.enter_context(tc.tile_pool(name="rp", bufs=6))
        fg = ctx.enter_context(tc.tile_pool(name="fg", bufs=5))
        tt = ctx.enter_context(tc.tile_pool(name="tt", bufs=5))
        small = ctx.enter_context(tc.tile_pool(name="small", bufs=2))
        cps = ctx.enter_context(tc.tile_pool(name="cps", bufs=6, space="PSUM"))
        tps = ctx.enter_context(tc.tile_pool(name="tps", bufs=2, space="PSUM"))

        ident = perm.tile([128, 128], F32, tag="ident")
        make_identity(nc, ident)

        # ---- phase A: gather seq embeddings, build embT[kc] = [128, 64, 31]
        embT = [perm.tile([128, BC, TW], F32, tag=f"embT{kc}", name=f"embT{kc}") for kc in (0, 1)]
        embTb = [perm.tile([128, TW, BC], BF16, tag=f"embTb{kc}", name=f"embTb{kc}") for kc in (0, 1)]
        for kc in (0, 1):
            nc.vector.memset(embT[kc][:, :, 0:PAD], 0.0)
            nc.gpsimd.memset(embTb[kc][:, 0:PAD, :], 0.0)
        for c in range(8):
            it = idxp.tile([128, 1], I32, tag="seqidx")
            nc.sync.dma_start(it[:], seq8[c, :, None])
            gt = gath.tile([128, D], F32, tag="embg")
            nc.gpsimd.indirect_dma_start(
                out=gt[:], out_offset=None, in_=item_emb[:],
                in_offset=bass.IndirectOffsetOnAxis(ap=it[:, :1], axis=0))
            for kc in (0, 1):
                tp = tps.tile([128, 128], F32, tag="tp")
                nc.tensor.transpose(tp[:], gt[:, kc * 128:(kc + 1) * 128], ident[:])
                nc.scalar.copy(embT[kc][:, 8 * c:8 * (c + 1), PAD:TW], tp[:])
                nc.scalar.copy(embTb[kc][:, PAD:TW, 8 * c:8 * (c + 1)].rearrange("p t b -> p b t"), tp[:])

        # ---- conv biases
        cb = perm.tile([128, 2, L], F32, tag="cb")
        nc.sync.dma_start(cb[:], convb[:])

        # ---- output accumulators o[c, b]
        oacc = [perm.tile([128, BC], F32, tag=f"oacc{cc}", name=f"oacc{cc}") for cc in (0, 1)]
        o3acc = [perm.tile([128, BC, S], F32, tag=f"o3acc{cc}", name=f"o3acc{cc}") for cc in (0, 1)]
        for cc in (0, 1):
            nc.vector.memset(o3acc[cc][:], 0.0)

        # user embedding -> uT chunks
        uidx = idxp.tile([BC, 1], I32, tag="uidx")
        nc.sync.dma_start(uidx[:], useri[:, None])
        ug = gath.tile([BC, D], F32, tag="ug")
        nc.gpsimd.indirect_dma_start(
            out=ug[:], out_offset=None, in_=user_emb[:],
            in_offset=bass.IndirectOffsetOnAxis(ap=uidx[:, :1], axis=0))
        catT = [oacc[0], oacc[1]]
        for kc in (0, 1):
            tp = tps.tile([128, 128], F32, tag="tp")
            nc.tensor.transpose(tp[:, :BC], ug[:, kc * 128:(kc + 1) * 128], ident[:BC, :BC])
            ut = small.tile([128, BC], F32, tag=f"ut{kc}")
            nc.any.tensor_copy(ut[:], tp[:, :BC])
            catT.append(ut)

        # W2 row gathers -> w2t[kc] = [128, 2048] (c on partitions, (b,n) free)
        w2t = [perm.tile([128, BC * N_TGT], F32, tag=f"w2t{kc}", name=f"w2t{kc}") for kc in (0, 1)]
        for ch in range(16):
            it = idxp.tile([128, 1], I32, tag="itemidx")
            nc.sync.dma_start(it[:], item16[ch, :, None])
            wg = gath.tile([128, D], F32, tag="w2g")
            nc.gpsimd.indirect_dma_start(
                out=wg[:], out_offset=None, in_=w2tab[:],
                in_offset=bass.IndirectOffsetOnAxis(ap=it[:, :1], axis=0))
            for kc in (0, 1):
                tp = tps.tile([128, 128], F32, tag="tp")
                nc.tensor.transpose(tp[:], wg[:, kc * 128:(kc + 1) * 128], ident[:])
                nc.scalar.copy(w2t[kc][:, 128 * ch:128 * (ch + 1)], tp[:])

        # ---- phase B: per-l conv + gates + triple fo-pool scan
        for l in range(L):
            wts = []
            for m in range(l + 1):
                w_t = wpool.tile([128, 2, D], BF16, tag="wt")
                nc.sync.dma_start(w_t[:], wt[TRI[l] + m].rearrange("(kc k) c -> k kc c", k=128))
                wts.append(w_t)
            pst = [[cps.tile([128, 512], F32, tag="cps", name=f"pst{l}_{i}_{h}")
                    for h in (0, 1)] for i in (0, 1)]
            for m in range(l + 1):
                for kc in (0, 1):
                    for cc in (0, 1):
                        lhs = wts[m][:, kc, cc * 128:(cc + 1) * 128]
                        for h in (0, 1):
                            # psum is t-major per half: col = 32*t + b. Taps with
                            # t < m are structurally zero -> write cols [32m, 512)
                            rhs = embTb[kc][:, PAD:PAD + T - m, 32 * h:32 * (h + 1)]
                            nc.tensor.matmul(
                                pst[cc][h][:, 32 * m:512],
                                lhsT=lhs, rhs=rhs,
                                start=(m == 0 and kc == 0),
                                stop=(m == l and kc == 1))
            fts, gts = [], []
            for cc in (0, 1):
                # r = relu(z + b);  f = sigmoid(r);  g = sigmoid(-r) = 1 - f
                f_t = fg.tile([128, BC, S], F32, tag="f", name=f"f{l}_{cc}")
                g_t = fg.tile([128, BC, S], F32, tag="g", name=f"g{l}_{cc}")
                nc.gpsimd.memset(f_t[:, :, 0:1], 0.0)
                nc.gpsimd.memset(g_t[:, :, 0:1], 0.0)
                for h in (0, 1):
                    r_t = rp.tile([128, 512], F32, tag="r")
                    nc.scalar.activation(r_t[:], pst[cc][h][:], AF.Relu,
                                         bias=cb[:, cc, l:l + 1], scale=1.0)
                    r3 = r_t[:].rearrange("p (t b) -> p t b", t=T)
                    f3 = f_t[:, 32 * h:32 * (h + 1), 1:S].rearrange("p b t -> p t b")
                    g3 = g_t[:, 32 * h:32 * (h + 1), 1:S].rearrange("p b t -> p t b")
                    nc.scalar.activation(f3, r3, AF.Sigmoid)
                    nc.scalar.activation(g3, r3, AF.Sigmoid, scale=-1.0)
                fts.append(f_t); gts.append(g_t)
            # interleave the two cc chains so Pool muls and DVE scans ping-pong
            curs = [None, None]
            for chain in range(N_L):
                fxs = [None, None]
                for cc in (0, 1):
                    fx = tt.tile([128, BC, S], F32, tag="fx", name=f"fx{l}_{cc}_{chain}")
                    xin = embT[cc][:, :, PAD - 1:TW] if chain == 0 else curs[cc][:]
                    nc.gpsimd.tensor_tensor(out=fx[:], in0=fts[cc][:], in1=xin, op=ALU.mult)
                    fxs[cc] = fx
                for cc in (0, 1):
                    hn = tt.tile([128, BC, S], F32, tag="hh", name=f"hh{l}_{cc}_{chain}")
                    nc.vector.tensor_tensor_scan(
                        out=hn[:].rearrange("p b t -> p (b t)"),
                        data0=gts[cc][:].rearrange("p b t -> p (b t)"),
                        data1=fxs[cc][:].rearrange("p b t -> p (b t)"),
                        initial=0.0, op0=ALU.mult, op1=ALU.add)
                    curs[cc] = hn
            for cc in (0, 1):
                nc.gpsimd.dma_start(o3acc[cc][:], curs[cc][:], accum_op=ALU.add)

        for cc in (0, 1):
            nc.vector.reduce_sum(oacc[cc][:], o3acc[cc][:], axis=mybir.AxisListType.X)

        # ---- phase C: head (gathers/transposes hoisted before conv)
        # z^T = fc1_w @ cat^T + b  -> [zc(2 chunks of 128), b(64)]
        f1w = perm.tile([128, 4, D], F32, tag="f1w")
        nc.sync.dma_start(f1w[:], fc1wt.rearrange("(kc k) c -> k kc c", k=128))
        f1b = perm.tile([128, 2], F32, tag="f1b")
        nc.sync.dma_start(f1b[:], fc1b[:])
        zT = []
        for cc in (0, 1):
            zp = tps.tile([128, BC], F32, tag="tp")
            for kc in range(4):
                nc.tensor.matmul(
                    zp[:], lhsT=f1w[:, kc, cc * 128:(cc + 1) * 128],
                    rhs=catT[kc][:],
                    start=(kc == 0), stop=(kc == 3))
            zt = small.tile([128, BC], F32, tag=f"zt{cc}")
            nc.scalar.activation(zt[:], zp[:], AF.Identity, bias=f1b[:, cc:cc + 1])
            zT.append(zt)

        # res[b,n] = sum_c w2t[c,(b,n)] * z[c,b]  (mul + ones-matmul partition sum)
        for kc in (0, 1):
            nc.gpsimd.tensor_tensor(
                out=w2t[kc][:].rearrange("p (b n) -> p b n", n=N_TGT),
                in0=w2t[kc][:].rearrange("p (b n) -> p b n", n=N_TGT),
                in1=zT[kc][:, :, None].to_broadcast((128, BC, N_TGT)),
                op=ALU.mult)
        ones = small.tile([128, 1], F32, tag="ones")
        nc.vector.memset(ones[:], 1.0)
        res_sb = small.tile([1, BC * N_TGT], F32, tag="ressb")
        for j in range(4):
            rj = tps.tile([1, 512], F32, tag="tp")
            for kc in (0, 1):
                nc.tensor.matmul(rj[:], lhsT=ones[:],
                                 rhs=w2t[kc][:, 512 * j:512 * (j + 1)],
                                 start=(kc == 0), stop=(kc == 1))
            nc.any.tensor_copy(res_sb[:, 512 * j:512 * (j + 1)], rj[:])
        nc.sync.dma_start(res.rearrange("b n -> (b n)")[None, :], res_sb[:])


_CACHED_NC = None


def build_nc():
    global _CACHED_NC
    if _CACHED_NC is not None:
        return _CACHED_NC
    nc = bacc.Bacc("TRN2", debug=False, enable_asserts=False)
    with tile.TileContext(nc) as tc:
        _build_kernel(nc, tc)
    nc.compile()
    _CACHED_NC = nc
    return nc


def make_in_maps(seq_var, user_var, item_var, item_emb, user_emb, conv_w,
                 conv_b, fc1_w, fc1_b, W2, b2):
    seq_var = np.asarray(seq_var).astype(np.int32)
    user_var = np.asarray(user_var).astype(np.int32)
    item_var = np.asarray(item_var).astype(np.int32)
    item_emb = np.ascontiguousarray(np.asarray(item_emb, dtype=np.float32))
    user_emb = np.ascontiguousarray(np.asarray(user_emb, dtype=np.float32))
    W2 = np.ascontiguousarray(np.asarray(W2, dtype=np.float32))
    conv_w = np.asarray(conv_w, dtype=np.float32)
    conv_b = np.ascontiguousarray(np.asarray(conv_b, dtype=np.float32))
    fc1_w = np.asarray(fc1_w, dtype=np.float32)
    fc1_b = np.ascontiguousarray(np.asarray(fc1_b, dtype=np.float32))

    # pack conv weights: block (l, m<=l) at TRI[l]+m = conv_w[l, m].T  ([d, c]), bf16
    import ml_dtypes
    wt_pack = np.empty((TRI[L], D, D), ml_dtypes.bfloat16)
    for l in range(L):
        for m in range(l + 1):
            wt_pack[TRI[l] + m] = conv_w[l, m].T.astype(ml_dtypes.bfloat16)
    fc1wt = np.ascontiguousarray(fc1_w.T)
    # convb_pack[c, cc, l] = conv_b[l, cc*128 + c];  fc1b_pack[c, cc] = fc1_b[cc*128+c]
    convb_pack = np.ascontiguousarray(conv_b.reshape(L, 2, 128).transpose(2, 1, 0))
    fc1b_pack = np.ascontiguousarray(fc1_b.reshape(2, 128).T)

    in_maps = []
    for c in range(N_CORES):
        sl = slice(c * BC, (c + 1) * BC)
        in_maps.append({
            "seq8": np.ascontiguousarray(seq_var[sl].reshape(8, 128)),
            "item16": np.ascontiguousarray(item_var[sl].reshape(16, 128)),
            "useri": np.ascontiguousarray(user_var[sl]),
            "item_emb": item_emb,
            "user_emb": user_emb,
            "w2tab": W2,
            "wt": wt_pack,
            "convb": convb_pack,
            "fc1wt": fc1wt,
            "fc1b": fc1b_pack,
        })
    return in_maps


def kernel(seq_var, user_var, item_var, item_emb, user_emb, conv_w, conv_b,
           fc1_w, fc1_b, W2, b2, _trace=False):
    from concourse import bass_utils
    nc = build_nc()
    in_maps = make_in_maps(seq_var, user_var, item_var, item_emb, user_emb,
                           conv_w, conv_b, fc1_w, fc1_b, W2, b2)
    r = bass_utils.run_bass_kernel_spmd(
        nc, in_maps, core_ids=list(range(N_CORES)), trace=_trace)
    out = np.concatenate([r.results[c]["res"] for c in range(N_CORES)], axis=0)
    b2 = np.asarray(b2, dtype=np.float32)
    item_var = np.asarray(item_var)
    out = out + b2[item_var][..., 0]
    if _trace:
        return out.astype(np.float32), r
    return out.astype(np.float32)



# revision 6
# speedup vs baseline: 1.5980x; 1.5980x over previous
"""Trainium2 Bass kernel for the QRNN-style recommender model.

Model (per batch row b):
  emb = item_emb[seq]                          # [T=16, D=256]
  conv_out[l,t,c] = sum_{m<=l} emb[t-m] @ W[l,m,c,:] + conv_b[l,c]   (L=16 causal convs)
  f = sigmoid(relu(conv_out))                  # forget gates
  h = fo-pool chain applied 3x over t (QRNN), x0 = emb
  o = sum over (l, t) of h                     # [D]
  z = [o, user_emb[user]] @ fc1_w.T + fc1_b    # [D]
  res[n] = W2[item[n]] . z + b2[item[n]]       # [N_TGT=32]

Sharding: data-parallel over batch B=512 across 8 cores (64 rows each);
all parameters/tables replicated; embedding gathers on-device via
indirect DMA.

Per-core implementation:
  conv: fp8(e4m3) DoubleRow matmuls (contraction 256 in one pass);
        emb scaled x256 and weights x64 on cast, undone by the
        activation scale 1/16384 in the gate pass.
  gates: one sigmoid pass per (l,cc,h) PSUM->SBUF; then
        f = max(sigmoid(z+b), 0.5) == sigmoid(relu(z+b)) via a DVE
        tensor_scalar max; g = 1-f via DVE tensor_scalar mult/add.
  fo-pool: explicit per-timestep recurrence, all 16 L-chains batched
        per op in bf16 (DVE 2x mode); the three chained applications
        advance as a staggered wavefront (h1/h2/h3 per t).
        f*x products and the o += h3 accumulation run on GpSimd.
  L is processed in two groups of 8 so the group-1 conv (74% of MACs)
  overlaps with the group-0 recurrence on DVE.
  head: fc1 via PE, per-row dot with gathered W2 rows via Pool mul +
        ones-vector PE partition-reduction.
"""
import numpy as np

import concourse.bass as bass
import concourse.mybir as mybir
import concourse.tile as tile
from concourse import bacc
from concourse.masks import make_identity

F32 = mybir.dt.float32
BF16 = mybir.dt.bfloat16
F8 = mybir.dt.float8e4
I32 = mybir.dt.int32
AF = mybir.ActivationFunctionType
ALU = mybir.AluOpType
DR = mybir.MatmulPerfMode.DoubleRow

# model dims (hardcoded per problem spec)
N_CORES = 8
B = 512
BC = B // N_CORES          # 64 rows per core
T = 16
L = 16
D = 256
N_TGT = 32
N_ITEMS = 200000
N_USERS = 100000
N_L = 3                    # fo-pool chain depth
TRI = [l * (l + 1) // 2 for l in range(L + 1)]  # block offsets for (l, m<=l)
WSCALE = 64.0              # fp8 weight scale
ESCALE = 256.0             # fp8 emb scale
LG = 2                     # l-groups
LH = L // LG               # 8 l's per group


def _build_kernel(nc, tc):
    seq8 = nc.dram_tensor("seq8", [8, 128], I32, kind="ExternalInput").ap()
    item16 = nc.dram_tensor("item16", [16, 128], I32, kind="ExternalInput").ap()
    useri = nc.dram_tensor("useri", [BC], I32, kind="ExternalInput").ap()
    item_emb = nc.dram_tensor("item_emb", [N_ITEMS, D], F32, kind="ExternalInput").ap()
    user_emb = nc.dram_tensor("user_emb", [N_USERS, D], F32, kind="ExternalInput").ap()
    w2tab = nc.dram_tensor("w2tab", [N_ITEMS, D], F32, kind="ExternalInput").ap()
    wt8 = nc.dram_tensor("wt8", [TRI[L], 128, 2, D], F8, kind="ExternalInput").ap()
    convb = nc.dram_tensor("convb", [128, 2, L], F32, kind="ExternalInput").ap()
    fc1wt = nc.dram_tensor("fc1wt", [2 * D, D], F32, kind="ExternalInput").ap()
    fc1b = nc.dram_tensor("fc1b", [128, 2], F32, kind="ExternalInput").ap()
    res = nc.dram_tensor("res", [BC, N_TGT], F32, kind="ExternalOutput").ap()

    import contextlib
    ctx = contextlib.ExitStack()
    with ctx:
        perm = ctx.enter_context(tc.tile_pool(name="perm", bufs=1))
        idxp = ctx.enter_context(tc.tile_pool(name="idxp", bufs=2))
        gath = ctx.enter_context(tc.tile_pool(name="gath", bufs=2))
        wpool = ctx.enter_context(tc.tile_pool(name="wpool", bufs=8))
        mmp = ctx.enter_context(tc.tile_pool(name="mmp", bufs=8))
        small = ctx.enter_context(tc.tile_pool(name="small", bufs=1))
        cps = ctx.enter_context(tc.tile_pool(name="cps", bufs=6, space="PSUM"))
        tps = ctx.enter_context(tc.tile_pool(name="tps", bufs=2, space="PSUM"))

        ident = perm.tile([128, 128], F32, tag="ident")
        make_identity(nc, ident)

        # ---- persistent tiles
        # emb8[p, kt, t, b] fp8 (conv rhs); x0[p, t, cc, b] bf16 (chain-1 x)
        emb8 = perm.tile([128, 2, T, BC], F8, tag="emb8")
        x0 = perm.tile([128, T, 2, BC], BF16, tag="x0")
        # gates: f/g[p, t, cc, l, b] bf16
        f_t = perm.tile([128, T, 2, L, BC], BF16, tag="f")
        g_t = perm.tile([128, T, 2, L, BC], BF16, tag="g")
        # o3[p, cc, l, b] bf16 accumulator
        o3 = perm.tile([128, 2, L, BC], BF16, tag="o3")
        nc.vector.memset(o3[:], 0.0)
        cb = perm.tile([128, 2, L], F32, tag="cb")
        nc.sync.dma_start(cb[:], convb[:])

        # ---- phase A: gather seq embeddings -> emb8 (fp8, x256) + x0 (bf16)
        for c in range(8):
            it = idxp.tile([128, 1], I32, tag="seqidx")
            nc.sync.dma_start(it[:], seq8[c, :, None])
            gt = gath.tile([128, D], F32, tag="embg")
            nc.gpsimd.indirect_dma_start(
                out=gt[:], out_offset=None, in_=item_emb[:],
                in_offset=bass.IndirectOffsetOnAxis(ap=it[:, :1], axis=0))
            for kc in (0, 1):
                tp = tps.tile([128, 128], F32, tag="tp")
                nc.tensor.transpose(tp[:], gt[:, kc * 128:(kc + 1) * 128], ident[:])
                # tp cols = (b_local 8) x (t 16)
                tpv = tp[:].rearrange("p (b t) -> p b t", b=8)
                nc.scalar.mul(
                    emb8[:, kc, :, 8 * c:8 * (c + 1)].rearrange("p t b -> p b t"),
                    tpv, ESCALE)
                nc.scalar.copy(
                    x0[:, :, kc, 8 * c:8 * (c + 1)].rearrange("p t b -> p b t"),
                    tpv)

        # ---- conv + gates + recurrence per l-group
        hseq = [None, None, None]  # h1/h2/h3 carry tiles per group
        for lg in range(LG):
            l0 = lg * LH
            # conv: fp8 DoubleRow matmuls, psum col = 32*t + b_half
            psts = []
            for l in range(l0, l0 + LH):
                w8s = []
                for m in range(l + 1):
                    w8 = wpool.tile([128, 2, D], F8, tag="w8")
                    nc.sync.dma_start(w8[:], wt8[TRI[l] + m])
                    w8s.append(w8)
                pst = [[cps.tile([128, 512], F32, tag="cps", name=f"pst{l}_{i}_{h}")
                        for h in (0, 1)] for i in (0, 1)]
                for m in range(l + 1):
                    for cc in (0, 1):
                        lhs = w8s[m][:, :, cc * 128:(cc + 1) * 128]
                        for h in (0, 1):
                            rhs = emb8[:, :, 0:T - m, 32 * h:32 * (h + 1)]
                            nc.tensor.matmul(
                                pst[cc][h][:, 32 * m:512],
                                lhsT=lhs, rhs=rhs, perf_mode=DR,
                                start=(m == 0), stop=(m == l))
                psts.append(pst)
            # gates: s = sigmoid(z/16384 + b) into f; f=max(s,.5); g=1-f
            for li, l in enumerate(range(l0, l0 + LH)):
                for cc in (0, 1):
                    for h in (0, 1):
                        nc.scalar.activation(
                            f_t[:, :, cc, l, 32 * h:32 * (h + 1)],
                            psts[li][cc][h][:].rearrange("p (t b) -> p t b", t=T),
                            AF.Sigmoid, bias=cb[:, cc, l:l + 1],
                            scale=1.0 / (WSCALE * ESCALE))
            fv = f_t[:, :, :, l0:l0 + LH, :].rearrange("p t c l b -> p t c (l b)")
            gv = g_t[:, :, :, l0:l0 + LH, :].rearrange("p t c l b -> p t c (l b)")
            nc.vector.tensor_scalar_max(fv, fv, 0.5)
            nc.vector.tensor_scalar(gv, fv, -1.0, 1.0, op0=ALU.mult, op1=ALU.add)

            # recurrence: h1/h2/h3 wavefront over t, all 8 l's batched
            h1 = perm.tile([128, 2, LH, BC], BF16, tag=f"h1_{lg}")
            h2 = perm.tile([128, 2, LH, BC], BF16, tag=f"h2_{lg}")
            h3 = perm.tile([128, 2, LH, BC], BF16, tag=f"h3_{lg}")
            ls = slice(l0, l0 + LH)
            for t in range(T):
                ft = f_t[:, t, :, ls, :]
                gt_ = g_t[:, t, :, ls, :]
                xb = x0[:, t, :, None, :].to_broadcast((128, 2, LH, BC))
                if t == 0:
                    nc.gpsimd.tensor_tensor(out=h1[:], in0=ft, in1=xb, op=ALU.mult)
                    nc.vector.tensor_tensor(out=h2[:], in0=ft, in1=h1[:], op=ALU.mult)
                    nc.vector.tensor_tensor(out=h3[:], in0=ft, in1=h2[:], op=ALU.mult)
                else:
                    m1 = mmp.tile([128, 2, LH, BC], BF16, tag="mm", name="m1")
                    nc.gpsimd.tensor_tensor(out=m1[:], in0=ft, in1=xb, op=ALU.mult)
                    mm = mmp.tile([128, 2, LH, BC], BF16, tag="mm", name="mm")
                    nc.vector.tensor_tensor(out=mm[:], in0=gt_, in1=h1[:], op=ALU.mult)
                    nc.vector.tensor_tensor(out=h1[:], in0=m1[:], in1=mm[:], op=ALU.add)
                    mm2 = mmp.tile([128, 2, LH, BC], BF16, tag="mm", name="mm2")
                    nc.vector.tensor_tensor(out=mm2[:], in0=ft, in1=h1[:], op=ALU.mult)
                    mm3 = mmp.tile([128, 2, LH, BC], BF16, tag="mm", name="mm3")
                    nc.vector.tensor_tensor(out=mm3[:], in0=gt_, in1=h2[:], op=ALU.mult)
                    nc.vector.tensor_tensor(out=h2[:], in0=mm2[:], in1=mm3[:], op=ALU.add)
                    mm4 = mmp.tile([128, 2, LH, BC], BF16, tag="mm", name="mm4")
                    nc.vector.tensor_tensor(out=mm4[:], in0=ft, in1=h2[:], op=ALU.mult)
                    mm5 = mmp.tile([128, 2, LH, BC], BF16, tag="mm", name="mm5")
                    nc.vector.tensor_tensor(out=mm5[:], in0=gt_, in1=h3[:], op=ALU.mult)
                    nc.vector.tensor_tensor(out=h3[:], in0=mm4[:], in1=mm5[:], op=ALU.add)
                nc.gpsimd.tensor_tensor(out=o3[:, :, ls, :], in0=o3[:, :, ls, :],
                                        in1=h3[:], op=ALU.add)

        # ---- head inputs (emitted late; run on idle engines during conv/steps)
        uidx = idxp.tile([BC, 1], I32, tag="uidx")
        nc.sync.dma_start(uidx[:], useri[:, None])
        ug = gath.tile([BC, D], F32, tag="ug")
        nc.gpsimd.indirect_dma_start(
            out=ug[:], out_offset=None, in_=user_emb[:],
            in_offset=bass.IndirectOffsetOnAxis(ap=uidx[:, :1], axis=0))
        uts = []
        for kc in (0, 1):
            tp = tps.tile([128, 128], F32, tag="tp")
            nc.tensor.transpose(tp[:, :BC], ug[:, kc * 128:(kc + 1) * 128], ident[:BC, :BC])
            ut = small.tile([128, BC], F32, tag=f"ut{kc}")
            nc.any.tensor_copy(ut[:], tp[:, :BC])
            uts.append(ut)

        # W2 row gathers -> w2t[kc] = [128, 2048] (c on partitions, (b,n) free)
        w2t = [perm.tile([128, BC * N_TGT], F32, tag=f"w2t{kc}", name=f"w2t{kc}")
               for kc in (0, 1)]
        for ch in range(16):
            it = idxp.tile([128, 1], I32, tag="itemidx")
            nc.sync.dma_start(it[:], item16[ch, :, None])
            wg = gath.tile([128, D], F32, tag="w2g")
            nc.gpsimd.indirect_dma_start(
                out=wg[:], out_offset=None, in_=w2tab[:],
                in_offset=bass.IndirectOffsetOnAxis(ap=it[:, :1], axis=0))
            for kc in (0, 1):
                tp = tps.tile([128, 128], F32, tag="tp")
                nc.tensor.transpose(tp[:], wg[:, kc * 128:(kc + 1) * 128], ident[:])
                nc.scalar.copy(w2t[kc][:, 128 * ch:128 * (ch + 1)], tp[:])

        # ---- o[c, b] = sum over (l, t): reduce o3 over l
        oacc = [perm.tile([128, BC], F32, tag=f"oacc{cc}", name=f"oacc{cc}")
                for cc in (0, 1)]
        for cc in (0, 1):
            nc.vector.reduce_sum(oacc[cc][:],
                                 o3[:, cc].rearrange("p l b -> p b l"),
                                 axis=mybir.AxisListType.X)

        # ---- head: z^T = fc1_w @ [o; u]^T + b  -> [zc(2 chunks of 128), b]
        f1w = perm.tile([128, 4, D], F32, tag="f1w")
        nc.sync.dma_start(f1w[:], fc1wt.rearrange("(kc k) c -> k kc c", k=128))
        f1b = perm.tile([128, 2], F32, tag="f1b")
        nc.sync.dma_start(f1b[:], fc1b[:])
        catT = [oacc[0], oacc[1], uts[0], uts[1]]
        zT = []
        for cc in (0, 1):
            zp = tps.tile([128, BC], F32, tag="tp")
            for kc in range(4):
                nc.tensor.matmul(
                    zp[:], lhsT=f1w[:, kc, cc * 128:(cc + 1) * 128],
                    rhs=catT[kc][:],
                    start=(kc == 0), stop=(kc == 3))
            zt = small.tile([128, BC], F32, tag=f"zt{cc}")
            nc.scalar.activation(zt[:], zp[:], AF.Identity, bias=f1b[:, cc:cc + 1])
            zT.append(zt)

        # res[b,n] = sum_c w2t[c,(b,n)] * z[c,b]  (mul + ones-matmul partition sum)
        for kc in (0, 1):
            nc.gpsimd.tensor_tensor(
                out=w2t[kc][:].rearrange("p (b n) -> p b n", n=N_TGT),
                in0=w2t[kc][:].rearrange("p (b n) -> p b n", n=N_TGT),
                in1=zT[kc][:, :, None].to_broadcast((128, BC, N_TGT)),
                op=ALU.mult)
        ones = small.tile([128, 1], F32, tag="ones")
        nc.vector.memset(ones[:], 1.0)
        res_sb = small.tile([1, BC * N_TGT], F32, tag="ressb")
        for j in range(4):
            rj = tps.tile([1, 512], F32, tag="tp")
            for kc in (0, 1):
                nc.tensor.matmul(rj[:], lhsT=ones[:],
                                 rhs=w2t[kc][:, 512 * j:512 * (j + 1)],
                                 start=(kc == 0), stop=(kc == 1))
            nc.any.tensor_copy(res_sb[:, 512 * j:512 * (j + 1)], rj[:])
        nc.sync.dma_start(res.rearrange("b n -> (b n)")[None, :], res_sb[:])


_CACHED_NC = None


def build_nc():
    global _CACHED_NC
    if _CACHED_NC is not None:
        return _CACHED_NC
    nc = bacc.Bacc("TRN2", debug=False, enable_asserts=False)
    with tile.TileContext(nc) as tc:
        _build_kernel(nc, tc)
    nc.compile()
    _CACHED_NC = nc
    return nc


def make_in_maps(seq_var, user_var, item_var, item_emb, user_emb, conv_w,
                 conv_b, fc1_w, fc1_b, W2, b2):
    seq_var = np.asarray(seq_var).astype(np.int32)
    user_var = np.asarray(user_var).astype(np.int32)
    item_var = np.asarray(item_var).astype(np.int32)
    item_emb = np.ascontiguousarray(np.asarray(item_emb, dtype=np.float32))
    user_emb = np.ascontiguousarray(np.asarray(user_emb, dtype=np.float32))
    W2 = np.ascontiguousarray(np.asarray(W2, dtype=np.float32))
    conv_w = np.asarray(conv_w, dtype=np.float32)
    conv_b = np.ascontiguousarray(np.asarray(conv_b, dtype=np.float32))
    fc1_w = np.asarray(fc1_w, dtype=np.float32)
    fc1_b = np.ascontiguousarray(np.asarray(fc1_b, dtype=np.float32))

    import ml_dtypes
    # wt8[tap, p, kt, c] = conv_w[l, m, c, kt*128 + p] * 64  (fp8 e4m3)
    wt8_pack = np.empty((TRI[L], 128, 2, D), ml_dtypes.float8_e4m3fn)
    for l in range(L):
        for m in range(l + 1):
            blk = (conv_w[l, m].T * WSCALE)          # [d, c]
            wt8_pack[TRI[l] + m] = blk.reshape(2, 128, D).transpose(1, 0, 2)
    fc1wt = np.ascontiguousarray(fc1_w.T)
    # convb_pack[c, cc, l] = conv_b[l, cc*128 + c];  fc1b_pack[c, cc] = fc1_b[cc*128+c]
    convb_pack = np.ascontiguousarray(conv_b.reshape(L, 2, 128).transpose(2, 1, 0))
    fc1b_pack = np.ascontiguousarray(fc1_b.reshape(2, 128).T)

    in_maps = []
    for c in range(N_CORES):
        sl = slice(c * BC, (c + 1) * BC)
        in_maps.append({
            "seq8": np.ascontiguousarray(seq_var[sl].reshape(8, 128)),
            "item16": np.ascontiguousarray(item_var[sl].reshape(16, 128)),
            "useri": np.ascontiguousarray(user_var[sl]),
            "item_emb": item_emb,
            "user_emb": user_emb,
            "w2tab": W2,
            "wt8": wt8_pack,
            "convb": convb_pack,
            "fc1wt": fc1wt,
            "fc1b": fc1b_pack,
        })
    return in_maps


def kernel(seq_var, user_var, item_var, item_emb, user_emb, conv_w, conv_b,
           fc1_w, fc1_b, W2, b2, _trace=False):
    from concourse import bass_utils
    nc = build_nc()
    in_maps = make_in_maps(seq_var, user_var, item_var, item_emb, user_emb,
                           conv_w, conv_b, fc1_w, fc1_b, W2, b2)
    r = bass_utils.run_bass_kernel_spmd(
        nc, in_maps, core_ids=list(range(N_CORES)), trace=_trace)
    out = np.concatenate([r.results[c]["res"] for c in range(N_CORES)], axis=0)
    b2 = np.asarray(b2, dtype=np.float32)
    item_var = np.asarray(item_var)
    out = out + b2[item_var][..., 0]
    if _trace:
        return out.astype(np.float32), r
    return out.astype(np.float32)


# revision 10
# speedup vs baseline: 1.7317x; 1.0837x over previous
"""Trainium2 Bass kernel for the QRNN-style recommender model.

Model (per batch row b):
  emb = item_emb[seq]                          # [T=16, D=256]
  conv_out[l,t,c] = sum_{m<=l} emb[t-m] @ W[l,m,c,:] + conv_b[l,c]   (L=16 causal convs)
  f = sigmoid(relu(conv_out))                  # forget gates
  h = fo-pool chain applied 3x over t (QRNN), x0 = emb
  o = sum over (l, t) of h                     # [D]
  z = [o, user_emb[user]] @ fc1_w.T + fc1_b    # [D]
  res[n] = W2[item[n]] . z + b2[item[n]]       # [N_TGT=32]

Sharding: data-parallel over batch B=512 across 8 cores (64 rows each);
all parameters/tables replicated; embedding gathers on-device via
indirect DMA.

Per-core implementation:
  conv: fp8(e4m3) DoubleRow matmuls (contraction 256 in one pass);
        emb scaled x256 and weights x64 on cast, undone by the
        activation scale 1/16384 in the gate pass.
  gates: one sigmoid pass per (l,cc,h) PSUM->SBUF; then
        f = max(sigmoid(z+b), 0.5) == sigmoid(relu(z+b)) via a DVE
        tensor_scalar max; g = 1-f via DVE tensor_scalar mult/add.
  fo-pool: explicit per-timestep recurrence, all 16 L-chains batched
        per op in bf16 (DVE 2x mode); the three chained applications
        advance as a staggered wavefront (h1/h2/h3 per t).
        f*x products and the o += h3 accumulation run on GpSimd.
  L is processed in two groups of 8 so the group-1 conv (74% of MACs)
  overlaps with the group-0 recurrence on DVE.
  head: fc1 via PE, per-row dot with gathered W2 rows via Pool mul +
        ones-vector PE partition-reduction.
"""
import numpy as np

import concourse.bass as bass
import concourse.mybir as mybir
import concourse.tile as tile
from concourse import bacc
from concourse.masks import make_identity

F32 = mybir.dt.float32
BF16 = mybir.dt.bfloat16
F8 = mybir.dt.float8e4
I32 = mybir.dt.int32
AF = mybir.ActivationFunctionType
ALU = mybir.AluOpType
DR = mybir.MatmulPerfMode.DoubleRow

# model dims (hardcoded per problem spec)
N_CORES = 8
B = 512
BC = B // N_CORES          # 64 rows per core
T = 16
L = 16
D = 256
N_TGT = 32
N_ITEMS = 200000
N_USERS = 100000
N_L = 3                    # fo-pool chain depth
TRI = [l * (l + 1) // 2 for l in range(L + 1)]  # block offsets for (l, m<=l)
WSCALE = 64.0              # fp8 weight scale
ESCALE = 256.0             # fp8 emb scale
LG = 2                     # l-groups
LH = L // LG               # 8 l's per group


def _build_kernel(nc, tc):
    seq8 = nc.dram_tensor("seq8", [8, 128], I32, kind="ExternalInput").ap()
    item16 = nc.dram_tensor("item16", [16, 128], I32, kind="ExternalInput").ap()
    useri = nc.dram_tensor("useri", [BC], I32, kind="ExternalInput").ap()
    item_emb = nc.dram_tensor("item_emb", [N_ITEMS, D], F32, kind="ExternalInput").ap()
    user_emb = nc.dram_tensor("user_emb", [N_USERS, D], F32, kind="ExternalInput").ap()
    w2tab = nc.dram_tensor("w2tab", [N_ITEMS, D], F32, kind="ExternalInput").ap()
    wt8 = nc.dram_tensor("wt8", [TRI[L], 128, 2, D], F8, kind="ExternalInput").ap()
    convb = nc.dram_tensor("convb", [128, 2, L], F32, kind="ExternalInput").ap()
    fc1wt = nc.dram_tensor("fc1wt", [2 * D, D], F32, kind="ExternalInput").ap()
    fc1b = nc.dram_tensor("fc1b", [128, 2], F32, kind="ExternalInput").ap()
    res = nc.dram_tensor("res", [BC, N_TGT], F32, kind="ExternalOutput").ap()

    import contextlib
    ctx = contextlib.ExitStack()
    with ctx:
        perm = ctx.enter_context(tc.tile_pool(name="perm", bufs=1))
        idxp = ctx.enter_context(tc.tile_pool(name="idxp", bufs=2))
        gath = ctx.enter_context(tc.tile_pool(name="gath", bufs=2))
        wpool = ctx.enter_context(tc.tile_pool(name="wpool", bufs=8))
        mmp = ctx.enter_context(tc.tile_pool(name="mmp", bufs=6))
        small = ctx.enter_context(tc.tile_pool(name="small", bufs=1))
        cps = ctx.enter_context(tc.tile_pool(name="cps", bufs=6, space="PSUM"))
        tps = ctx.enter_context(tc.tile_pool(name="tps", bufs=2, space="PSUM"))

        ident = perm.tile([128, 128], F32, tag="ident")
        make_identity(nc, ident)

        # ---- persistent tiles
        # emb8[p, kt, t, b] fp8 (conv rhs); x0[p, t, cc, b] bf16 (chain-1 x)
        emb8 = perm.tile([128, 2, T, BC], F8, tag="emb8")
        x0 = perm.tile([128, T, 2, BC], BF16, tag="x0")
        # gates: f/g[p, t, lg, cc, l8, b] bf16 — lg-major so each (t, lg)
        # step slice is one contiguous 2048-col block (DVE 2x mode)
        f_t = perm.tile([128, T, LG, 2, LH, BC], BF16, tag="f")
        g_t = perm.tile([128, T, LG, 2, LH, BC], BF16, tag="g")
        # o3[p, lg, cc, l8, b] bf16 accumulator
        o3 = perm.tile([128, LG, 2, LH, BC], BF16, tag="o3")
        nc.vector.memset(o3[:], 0.0)
        cb = perm.tile([128, 2, L], F32, tag="cb")
        nc.sync.dma_start(cb[:], convb[:])

        # ---- phase A: gather seq embeddings -> emb8 (fp8, x256) + x0 (bf16)
        for c in range(8):
            it = idxp.tile([128, 1], I32, tag="seqidx")
            nc.sync.dma_start(it[:], seq8[c, :, None])
            gt = gath.tile([128, D], F32, tag="embg")
            nc.gpsimd.indirect_dma_start(
                out=gt[:], out_offset=None, in_=item_emb[:],
                in_offset=bass.IndirectOffsetOnAxis(ap=it[:, :1], axis=0))
            for kc in (0, 1):
                tp = tps.tile([128, 128], F32, tag="tp")
                nc.tensor.transpose(tp[:], gt[:, kc * 128:(kc + 1) * 128], ident[:])
                # tp cols = (b_local 8) x (t 16)
                tpv = tp[:].rearrange("p (b t) -> p b t", b=8)
                nc.scalar.mul(
                    emb8[:, kc, :, 8 * c:8 * (c + 1)].rearrange("p t b -> p b t"),
                    tpv, ESCALE)
                nc.scalar.copy(
                    x0[:, :, kc, 8 * c:8 * (c + 1)].rearrange("p t b -> p b t"),
                    tpv)

        # ---- head inputs (emitted early; run on idle engines before/during conv)
        uidx = idxp.tile([BC, 1], I32, tag="uidx")
        nc.sync.dma_start(uidx[:], useri[:, None])
        ug = gath.tile([BC, D], F32, tag="ug")
        nc.gpsimd.indirect_dma_start(
            out=ug[:], out_offset=None, in_=user_emb[:],
            in_offset=bass.IndirectOffsetOnAxis(ap=uidx[:, :1], axis=0))
        uts = []
        for kc in (0, 1):
            tp = tps.tile([128, 128], F32, tag="tp")
            nc.tensor.transpose(tp[:, :BC], ug[:, kc * 128:(kc + 1) * 128], ident[:BC, :BC])
            ut = small.tile([128, BC], F32, tag=f"ut{kc}")
            nc.any.tensor_copy(ut[:], tp[:, :BC])
            uts.append(ut)

        # W2 row gathers -> w2t[kc] = [128, 2048] (c on partitions, (b,n) free)
        w2t = [perm.tile([128, BC * N_TGT], F32, tag=f"w2t{kc}", name=f"w2t{kc}")
               for kc in (0, 1)]
        for ch in range(16):
            it = idxp.tile([128, 1], I32, tag="itemidx")
            nc.sync.dma_start(it[:], item16[ch, :, None])
            wg = gath.tile([128, D], F32, tag="w2g")
            nc.gpsimd.indirect_dma_start(
                out=wg[:], out_offset=None, in_=w2tab[:],
                in_offset=bass.IndirectOffsetOnAxis(ap=it[:, :1], axis=0))
            for kc in (0, 1):
                tp = tps.tile([128, 128], F32, tag="tp")
                nc.tensor.transpose(tp[:], wg[:, kc * 128:(kc + 1) * 128], ident[:])
                nc.scalar.copy(w2t[kc][:, 128 * ch:128 * (ch + 1)], tp[:])

        # ---- conv + gates + recurrence per l-group
        for lg in range(LG):
            l0 = lg * LH
            # conv: fp8 DoubleRow matmuls, psum col = 32*t + b_half
            psts = []
            for l in range(l0, l0 + LH):
                w8s = []
                for m in range(l + 1):
                    w8 = wpool.tile([128, 2, D], F8, tag="w8")
                    nc.sync.dma_start(w8[:], wt8[TRI[l] + m])
                    w8s.append(w8)
                pst = [[cps.tile([128, 512], F32, tag="cps", name=f"pst{l}_{i}_{h}")
                        for h in (0, 1)] for i in (0, 1)]
                for m in range(l + 1):
                    for cc in (0, 1):
                        lhs = w8s[m][:, :, cc * 128:(cc + 1) * 128]
                        for h in (0, 1):
                            rhs = emb8[:, :, 0:T - m, 32 * h:32 * (h + 1)]
                            nc.tensor.matmul(
                                pst[cc][h][:, 32 * m:512],
                                lhsT=lhs, rhs=rhs, perf_mode=DR,
                                start=(m == 0), stop=(m == l))
                psts.append(pst)
            # gates: s = sigmoid(z/16384 + b) into f; f=max(s,.5); g=1-f
            for li, l in enumerate(range(l0, l0 + LH)):
                for cc in (0, 1):
                    for h in (0, 1):
                        nc.scalar.activation(
                            f_t[:, :, lg, cc, li, 32 * h:32 * (h + 1)],
                            psts[li][cc][h][:].rearrange("p (t b) -> p t b", t=T),
                            AF.Sigmoid, bias=cb[:, cc, l:l + 1],
                            scale=1.0 / (WSCALE * ESCALE))
            fv = f_t[:, :, lg].rearrange("p t c l b -> p t (c l b)")
            gv = g_t[:, :, lg].rearrange("p t c l b -> p t (c l b)")
            nc.vector.tensor_scalar_max(fv, fv, 0.5)
            nc.vector.tensor_scalar(gv, fv, -1.0, 1.0, op0=ALU.mult, op1=ALU.add)

            # recurrence: h1/h2/h3 wavefront over t, all 8 l's batched; every
            # step slice is a contiguous [128, 2048] block (DVE 2x mode)
            h1 = perm.tile([128, 2, LH, BC], BF16, tag=f"h1_{lg}", name=f"h1_{lg}")
            h2 = perm.tile([128, 2, LH, BC], BF16, tag=f"h2_{lg}", name=f"h2_{lg}")
            h3 = perm.tile([128, 2, LH, BC], BF16, tag=f"h3_{lg}", name=f"h3_{lg}")
            h1f, h2f, h3f = (h[:].rearrange("p c l b -> p (c l b)")
                             for h in (h1, h2, h3))
            for t in range(T):
                ft = f_t[:, t, lg].rearrange("p c l b -> p (c l b)")
                gt_ = g_t[:, t, lg].rearrange("p c l b -> p (c l b)")
                ft4 = f_t[:, t, lg]
                xb = x0[:, t, :, None, :].to_broadcast((128, 2, LH, BC))
                if t == 0:
                    nc.vector.tensor_tensor(out=h1[:], in0=ft4, in1=xb, op=ALU.mult)
                    nc.vector.tensor_tensor(out=h2f, in0=ft, in1=h1f, op=ALU.mult)
                    nc.vector.tensor_tensor(out=h3f, in0=ft, in1=h2f, op=ALU.mult)
                else:
                    m1 = mmp.tile([128, 2, LH, BC], BF16, tag="mm", name="m1")
                    nc.vector.tensor_tensor(out=m1[:], in0=ft4, in1=xb, op=ALU.mult)
                    m1f = m1[:].rearrange("p c l b -> p (c l b)")
                    mm = mmp.tile([128, 1024], BF16, tag="mm", name="mm")
                    nc.vector.tensor_tensor(out=mm[:], in0=gt_, in1=h1f, op=ALU.mult)
                    nc.vector.tensor_tensor(out=h1f, in0=m1f, in1=mm[:], op=ALU.add)
                    mm2 = mmp.tile([128, 1024], BF16, tag="mm", name="mm2")
                    nc.vector.tensor_tensor(out=mm2[:], in0=ft, in1=h1f, op=ALU.mult)
                    mm3 = mmp.tile([128, 1024], BF16, tag="mm", name="mm3")
                    nc.vector.tensor_tensor(out=mm3[:], in0=gt_, in1=h2f, op=ALU.mult)
                    nc.vector.tensor_tensor(out=h2f, in0=mm2[:], in1=mm3[:], op=ALU.add)
                    mm4 = mmp.tile([128, 1024], BF16, tag="mm", name="mm4")
                    nc.vector.tensor_tensor(out=mm4[:], in0=ft, in1=h2f, op=ALU.mult)
                    mm5 = mmp.tile([128, 1024], BF16, tag="mm", name="mm5")
                    nc.vector.tensor_tensor(out=mm5[:], in0=gt_, in1=h3f, op=ALU.mult)
                    nc.vector.tensor_tensor(out=h3f, in0=mm4[:], in1=mm5[:], op=ALU.add)
                o3f = o3[:, lg].rearrange("p c l b -> p (c l b)")
                nc.gpsimd.tensor_tensor(out=o3f, in0=o3f, in1=h3f, op=ALU.add)

        # ---- o[c, b] = sum over (lg, l8, t)
        oacc = [perm.tile([128, BC], F32, tag=f"oacc{cc}", name=f"oacc{cc}")
                for cc in (0, 1)]
        for cc in (0, 1):
            osum = small.tile([128, LH, BC], F32, tag=f"osum{cc}")
            nc.vector.tensor_tensor(out=osum[:], in0=o3[:, 0, cc],
                                    in1=o3[:, 1, cc], op=ALU.add)
            nc.vector.reduce_sum(oacc[cc][:],
                                 osum[:].rearrange("p l b -> p b l"),
                                 axis=mybir.AxisListType.X)

        # ---- head: z^T = fc1_w @ [o; u]^T + b  -> [zc(2 chunks of 128), b]
        f1w = perm.tile([128, 4, D], F32, tag="f1w")
        nc.sync.dma_start(f1w[:], fc1wt.rearrange("(kc k) c -> k kc c", k=128))
        f1b = perm.tile([128, 2], F32, tag="f1b")
        nc.sync.dma_start(f1b[:], fc1b[:])
        catT = [oacc[0], oacc[1], uts[0], uts[1]]
        zT = []
        for cc in (0, 1):
            zp = tps.tile([128, BC], F32, tag="tp")
            for kc in range(4):
                nc.tensor.matmul(
                    zp[:], lhsT=f1w[:, kc, cc * 128:(cc + 1) * 128],
                    rhs=catT[kc][:],
                    start=(kc == 0), stop=(kc == 3))
            zt = small.tile([128, BC], F32, tag=f"zt{cc}")
            nc.scalar.activation(zt[:], zp[:], AF.Identity, bias=f1b[:, cc:cc + 1])
            zT.append(zt)

        # res[b,n] = sum_c w2t[c,(b,n)] * z[c,b]  (mul + ones-matmul partition sum)
        for kc in (0, 1):
            nc.gpsimd.tensor_tensor(
                out=w2t[kc][:].rearrange("p (b n) -> p b n", n=N_TGT),
                in0=w2t[kc][:].rearrange("p (b n) -> p b n", n=N_TGT),
                in1=zT[kc][:, :, None].to_broadcast((128, BC, N_TGT)),
                op=ALU.mult)
        ones = small.tile([128, 1], F32, tag="ones")
        nc.vector.memset(ones[:], 1.0)
        res_sb = small.tile([1, BC * N_TGT], F32, tag="ressb")
        for j in range(4):
            rj = tps.tile([1, 512], F32, tag="tp")
            for kc in (0, 1):
                nc.tensor.matmul(rj[:], lhsT=ones[:],
                                 rhs=w2t[kc][:, 512 * j:512 * (j + 1)],
                                 start=(kc == 0), stop=(kc == 1))
            nc.any.tensor_copy(res_sb[:, 512 * j:512 * (j + 1)], rj[:])
        nc.sync.dma_start(res.rearrange("b n -> (b n)")[None, :], res_sb[:])


_CACHED_NC = None


def build_nc():
    global _CACHED_NC
    if _CACHED_NC is not None:
        return _CACHED_NC
    nc = bacc.Bacc("TRN2", debug=False, enable_asserts=False)
    with tile.TileContext(nc) as tc:
        _build_kernel(nc, tc)
    nc.compile()
    _CACHED_NC = nc
    return nc


def make_in_maps(seq_var, user_var, item_var, item_emb, user_emb, conv_w,
                 conv_b, fc1_w, fc1_b, W2, b2):
    seq_var = np.asarray(seq_var).astype(np.int32)
    user_var = np.asarray(user_var).astype(np.int32)
    item_var = np.asarray(item_var).astype(np.int32)
    item_emb = np.ascontiguousarray(np.asarray(item_emb, dtype=np.float32))
    user_emb = np.ascontiguousarray(np.asarray(user_emb, dtype=np.float32))
    W2 = np.ascontiguousarray(np.asarray(W2, dtype=np.float32))
    conv_w = np.asarray(conv_w, dtype=np.float32)
    conv_b = np.ascontiguousarray(np.asarray(conv_b, dtype=np.float32))
    fc1_w = np.asarray(fc1_w, dtype=np.float32)
    fc1_b = np.ascontiguousarray(np.asarray(fc1_b, dtype=np.float32))

    import ml_dtypes
    # wt8[tap, p, kt, c] = conv_w[l, m, c, kt*128 + p] * 64  (fp8 e4m3)
    wt8_pack = np.empty((TRI[L], 128, 2, D), ml_dtypes.float8_e4m3fn)
    for l in range(L):
        for m in range(l + 1):
            blk = (conv_w[l, m].T * WSCALE)          # [d, c]
            wt8_pack[TRI[l] + m] = blk.reshape(2, 128, D).transpose(1, 0, 2)
    fc1wt = np.ascontiguousarray(fc1_w.T)
    # convb_pack[c, cc, l] = conv_b[l, cc*128 + c];  fc1b_pack[c, cc] = fc1_b[cc*128+c]
    convb_pack = np.ascontiguousarray(conv_b.reshape(L, 2, 128).transpose(2, 1, 0))
    fc1b_pack = np.ascontiguousarray(fc1_b.reshape(2, 128).T)

    in_maps = []
    for c in range(N_CORES):
        sl = slice(c * BC, (c + 1) * BC)
        in_maps.append({
            "seq8": np.ascontiguousarray(seq_var[sl].reshape(8, 128)),
            "item16": np.ascontiguousarray(item_var[sl].reshape(16, 128)),
            "useri": np.ascontiguousarray(user_var[sl]),
            "item_emb": item_emb,
            "user_emb": user_emb,
            "w2tab": W2,
            "wt8": wt8_pack,
            "convb": convb_pack,
            "fc1wt": fc1wt,
            "fc1b": fc1b_pack,
        })
    return in_maps


def kernel(seq_var, user_var, item_var, item_emb, user_emb, conv_w, conv_b,
           fc1_w, fc1_b, W2, b2, _trace=False):
    from concourse import bass_utils
    nc = build_nc()
    in_maps = make_in_maps(seq_var, user_var, item_var, item_emb, user_emb,
                           conv_w, conv_b, fc1_w, fc1_b, W2, b2)
    r = bass_utils.run_bass_kernel_spmd(
        nc, in_maps, core_ids=list(range(N_CORES)), trace=_trace)
    out = np.concatenate([r.results[c]["res"] for c in range(N_CORES)], axis=0)
    b2 = np.asarray(b2, dtype=np.float32)
    item_var = np.asarray(item_var)
    out = out + b2[item_var][..., 0]
    if _trace:
        return out.astype(np.float32), r
    return out.astype(np.float32)


# revision 15
# speedup vs baseline: 2.0044x; 1.1575x over previous
"""Trainium2 Bass kernel for the QRNN-style recommender model.

Model (per batch row b):
  emb = item_emb[seq]                          # [T=16, D=256]
  conv_out[l,t,c] = sum_{m<=l} emb[t-m] @ W[l,m,c,:] + conv_b[l,c]   (L=16 causal convs)
  f = sigmoid(relu(conv_out))                  # forget gates
  h = fo-pool chain applied 3x over t (QRNN), x0 = emb
  o = sum over (l, t) of h                     # [D]
  z = [o, user_emb[user]] @ fc1_w.T + fc1_b    # [D]
  res[n] = W2[item[n]] . z + b2[item[n]]       # [N_TGT=32]

Sharding: data-parallel over batch B=512 across 8 cores (64 rows each);
all parameters/tables replicated; embedding gathers on-device via
indirect DMA.

Per-core implementation:
  conv: fp8(e4m3) DoubleRow matmuls (contraction 256 in one pass);
        emb scaled x256 and weights x64 on cast, undone by the
        activation scale 1/16384 in the gate pass.
  gates: one sigmoid pass per (l,cc,h) PSUM->SBUF; then
        f = max(sigmoid(z+b), 0.5) == sigmoid(relu(z+b)) via a DVE
        tensor_scalar max; g = 1-f via DVE tensor_scalar mult/add.
  fo-pool: explicit per-timestep recurrence, all 16 L-chains batched
        per op in bf16 (DVE 2x mode); the three chained applications
        advance as a staggered wavefront (h1/h2/h3 per t).
        f*x products and the o += h3 accumulation run on GpSimd.
  L is processed in two groups of 8 so the group-1 conv (74% of MACs)
  overlaps with the group-0 recurrence on DVE.
  head: fc1 via PE, per-row dot with gathered W2 rows via Pool mul +
        ones-vector PE partition-reduction.
"""
import numpy as np

import concourse.bass as bass
import concourse.mybir as mybir
import concourse.tile as tile
from concourse import bacc
from concourse.masks import make_identity

F32 = mybir.dt.float32
BF16 = mybir.dt.bfloat16
F8 = mybir.dt.float8e4
I32 = mybir.dt.int32
AF = mybir.ActivationFunctionType
ALU = mybir.AluOpType
DR = mybir.MatmulPerfMode.DoubleRow

# model dims (hardcoded per problem spec)
N_CORES = 8
B = 512
BC = B // N_CORES          # 64 rows per core
T = 16
L = 16
D = 256
N_TGT = 32
N_ITEMS = 200000
N_USERS = 100000
N_L = 3                    # fo-pool chain depth
TRI = [l * (l + 1) // 2 for l in range(L + 1)]  # block offsets for (l, m<=l)
WSCALE = 64.0              # fp8 weight scale
ESCALE = 256.0             # fp8 emb scale
LG = 2                     # l-groups
LH = L // LG               # 8 l's per group


def _build_kernel(nc, tc):
    seq8 = nc.dram_tensor("seq8", [8, 128], I32, kind="ExternalInput").ap()
    item16 = nc.dram_tensor("item16", [16, 128], I32, kind="ExternalInput").ap()
    useri = nc.dram_tensor("useri", [BC], I32, kind="ExternalInput").ap()
    item_emb = nc.dram_tensor("item_emb", [N_ITEMS, D], F32, kind="ExternalInput").ap()
    user_emb = nc.dram_tensor("user_emb", [N_USERS, D], F32, kind="ExternalInput").ap()
    w2tab = nc.dram_tensor("w2tab", [N_ITEMS, D], F32, kind="ExternalInput").ap()
    wt8 = nc.dram_tensor("wt8", [TRI[L], 128, 2, D], F8, kind="ExternalInput").ap()
    convb = nc.dram_tensor("convb", [128, 2, L], F32, kind="ExternalInput").ap()
    fc1wt = nc.dram_tensor("fc1wt", [2 * D, D], F32, kind="ExternalInput").ap()
    fc1b = nc.dram_tensor("fc1b", [128, 2], F32, kind="ExternalInput").ap()
    res = nc.dram_tensor("res", [BC, N_TGT], F32, kind="ExternalOutput").ap()

    import contextlib
    ctx = contextlib.ExitStack()
    with ctx:
        perm = ctx.enter_context(tc.tile_pool(name="perm", bufs=1))
        idxp = ctx.enter_context(tc.tile_pool(name="idxp", bufs=2))
        gath = ctx.enter_context(tc.tile_pool(name="gath", bufs=2))
        wpool = ctx.enter_context(tc.tile_pool(name="wpool", bufs=8))
        mmp = ctx.enter_context(tc.tile_pool(name="mmp", bufs=6))
        small = ctx.enter_context(tc.tile_pool(name="small", bufs=1))
        cps = ctx.enter_context(tc.tile_pool(name="cps", bufs=3, space="PSUM"))
        tps = ctx.enter_context(tc.tile_pool(name="tps", bufs=2, space="PSUM"))

        ident = perm.tile([128, 128], F32, tag="ident")
        make_identity(nc, ident)

        # ---- persistent tiles
        # emb8[p, kt, t, b] fp8 (conv rhs); x0[p, t, cc, b] bf16 (chain-1 x)
        emb8 = perm.tile([128, 2, T, BC], F8, tag="emb8")
        x0 = perm.tile([128, T, 2, BC], BF16, tag="x0")
        # gates: f/g[p, t, lg, cc, l8, b] bf16 — lg-major so each (t, lg)
        # step slice is one contiguous 2048-col block (DVE 2x mode)
        f_t = perm.tile([128, T, LG, 2, LH, BC], BF16, tag="f")
        g_t = perm.tile([128, T, LG, 2, LH, BC], BF16, tag="g")
        # o3[p, lg, cc, l8, b] bf16 accumulator
        o3 = perm.tile([128, LG, 2, LH, BC], BF16, tag="o3")
        nc.vector.memset(o3[:], 0.0)
        cb = perm.tile([128, 2, L], F32, tag="cb")
        nc.sync.dma_start(cb[:], convb[:])

        # ---- phase A: gather seq embeddings -> emb8 (fp8, x256) + x0 (bf16)
        tpvs = []
        for c in range(8):
            it = idxp.tile([128, 1], I32, tag="seqidx")
            nc.sync.dma_start(it[:], seq8[c, :, None])
            gt = gath.tile([128, D], F32, tag="embg")
            nc.gpsimd.indirect_dma_start(
                out=gt[:], out_offset=None, in_=item_emb[:],
                in_offset=bass.IndirectOffsetOnAxis(ap=it[:, :1], axis=0))
            for kc in (0, 1):
                tp = tps.tile([128, 128], F32, tag="tp", name=f"tp{c}_{kc}")
                nc.tensor.transpose(tp[:], gt[:, kc * 128:(kc + 1) * 128], ident[:])
                # tp cols = (b_local 8) x (t 16); emb8 (conv-critical) on Scalar
                tpv = tp[:].rearrange("p (b t) -> p b t", b=8)
                nc.scalar.mul(
                    emb8[:, kc, :, 8 * c:8 * (c + 1)].rearrange("p t b -> p b t"),
                    tpv, ESCALE)
                # x0 (needed later) on idle DVE
                nc.vector.tensor_scalar_mul(
                    x0[:, :, kc, 8 * c:8 * (c + 1)].rearrange("p t b -> p b t"),
                    tpv, 1.0)

        w2t = [perm.tile([128, BC * N_TGT], F32, tag=f"w2t{kc}", name=f"w2t{kc}")
               for kc in (0, 1)]
        uts = []

        def emit_head_inputs():
            # head gathers/transposes/copies; emitted after the lg0 sigmoids so
            # the Scalar copies don't delay the gate pass
            uidx = idxp.tile([BC, 1], I32, tag="uidx")
            nc.sync.dma_start(uidx[:], useri[:, None])
            ug = gath.tile([BC, D], F32, tag="ug")
            nc.gpsimd.indirect_dma_start(
                out=ug[:], out_offset=None, in_=user_emb[:],
                in_offset=bass.IndirectOffsetOnAxis(ap=uidx[:, :1], axis=0))
            for kc in (0, 1):
                tp = tps.tile([128, 128], F32, tag="tp", name=f"tpu{kc}")
                nc.tensor.transpose(tp[:, :BC], ug[:, kc * 128:(kc + 1) * 128],
                                    ident[:BC, :BC])
                ut = small.tile([128, BC], F32, tag=f"ut{kc}", name=f"ut{kc}")
                nc.any.tensor_copy(ut[:], tp[:, :BC])
                uts.append(ut)
            # W2 row gathers -> w2t[kc] = [128, 2048] (c on partitions, (b,n) free)
            for ch in range(16):
                it = idxp.tile([128, 1], I32, tag="itemidx")
                nc.sync.dma_start(it[:], item16[ch, :, None])
                wg = gath.tile([128, D], F32, tag="w2g")
                nc.gpsimd.indirect_dma_start(
                    out=wg[:], out_offset=None, in_=w2tab[:],
                    in_offset=bass.IndirectOffsetOnAxis(ap=it[:, :1], axis=0))
                for kc in (0, 1):
                    tp = tps.tile([128, 128], F32, tag="tp", name=f"tpw{ch}_{kc}")
                    nc.tensor.transpose(tp[:], wg[:, kc * 128:(kc + 1) * 128], ident[:])
                    nc.scalar.copy(w2t[kc][:, 128 * ch:128 * (ch + 1)], tp[:])

        # ---- conv + gates + recurrence per l-group
        for lg in range(LG):
            l0 = lg * LH
            # conv: fp8 DoubleRow matmuls; psum tile [128, 1024] spans both
            # b-halves (col = 512*h + 32*t + b), one matmul per (l, m, cc)
            psts = []
            for l in range(l0, l0 + LH):
                w8s = []
                for m in range(l + 1):
                    w8 = wpool.tile([128, 2, D], F8, tag="w8")
                    nc.sync.dma_start(w8[:], wt8[TRI[l] + m])
                    w8s.append(w8)
                pst = [cps.tile([128, 1024], F32, tag="cps", name=f"pst{l}_{i}")
                       for i in (0, 1)]
                for m in range(l + 1):
                    for cc in (0, 1):
                        lhs = w8s[m][:, :, cc * 128:(cc + 1) * 128]
                        for h in (0, 1):
                            rhs = emb8[:, :, 0:T - m, 32 * h:32 * (h + 1)]
                            nc.tensor.matmul(
                                pst[cc][:, 512 * h + 32 * m:512 * (h + 1)],
                                lhsT=lhs, rhs=rhs, perf_mode=DR,
                                start=(m == 0), stop=(m == l))
                psts.append([p_[:].rearrange("p (h t b) -> p t h b", h=2, b=32)
                             for p_ in pst])
            # gates: s = sigmoid(z/16384 + b) into f; f=max(s,.5); g=1-f
            for li, l in enumerate(range(l0, l0 + LH)):
                for cc in (0, 1):
                    nc.scalar.activation(
                        f_t[:, :, lg, cc, li, :].rearrange(
                            "p t (h b) -> p t h b", h=2),
                        psts[li][cc],
                        AF.Sigmoid, bias=cb[:, cc, l:l + 1],
                        scale=1.0 / (WSCALE * ESCALE))
            if lg == 0:
                emit_head_inputs()
            fv = f_t[:, :, lg].rearrange("p t c l b -> p t (c l b)")
            gv = g_t[:, :, lg].rearrange("p t c l b -> p t (c l b)")
            nc.vector.tensor_scalar_max(fv, fv, 0.5)
            nc.vector.tensor_scalar(gv, fv, -1.0, 1.0, op0=ALU.mult, op1=ALU.add)

            # recurrence: h1/h2/h3 wavefront over t, all 8 l's batched; every
            # step slice is a contiguous [128, 1024] block (DVE 2x mode).
            # h3 is double-buffered so the trailing o3 DMA-accum read never
            # stalls the next step's h3 write.
            h1 = perm.tile([128, 2, LH, BC], BF16, tag=f"h1_{lg}", name=f"h1_{lg}")
            h2 = perm.tile([128, 2, LH, BC], BF16, tag=f"h2_{lg}", name=f"h2_{lg}")
            h3p = [perm.tile([128, 2, LH, BC], BF16, tag=f"h3_{lg}_{i}",
                             name=f"h3_{lg}_{i}") for i in (0, 1)]
            h1f, h2f = (h[:].rearrange("p c l b -> p (c l b)") for h in (h1, h2))
            h3pf = [h[:].rearrange("p c l b -> p (c l b)") for h in h3p]
            o3f = o3[:, lg].rearrange("p c l b -> p (c l b)")
            for t in range(T):
                ft = f_t[:, t, lg].rearrange("p c l b -> p (c l b)")
                gt_ = g_t[:, t, lg].rearrange("p c l b -> p (c l b)")
                ft4 = f_t[:, t, lg]
                xb = x0[:, t, :, None, :].to_broadcast((128, 2, LH, BC))
                h3f, h3fprev = h3pf[t % 2], h3pf[1 - t % 2]
                if t == 0:
                    nc.vector.tensor_tensor(out=h1[:], in0=ft4, in1=xb, op=ALU.mult)
                    nc.vector.tensor_tensor(out=h2f, in0=ft, in1=h1f, op=ALU.mult)
                    nc.vector.tensor_tensor(out=h3f, in0=ft, in1=h2f, op=ALU.mult)
                else:
                    m1 = mmp.tile([128, 2, LH, BC], BF16, tag="mm", name="m1")
                    nc.vector.tensor_tensor(out=m1[:], in0=ft4, in1=xb, op=ALU.mult)
                    m1f = m1[:].rearrange("p c l b -> p (c l b)")
                    mm = mmp.tile([128, 1024], BF16, tag="mm", name="mm")
                    nc.vector.tensor_tensor(out=mm[:], in0=gt_, in1=h1f, op=ALU.mult)
                    nc.vector.tensor_tensor(out=h1f, in0=m1f, in1=mm[:], op=ALU.add)
                    mm2 = mmp.tile([128, 1024], BF16, tag="mm", name="mm2")
                    nc.vector.tensor_tensor(out=mm2[:], in0=ft, in1=h1f, op=ALU.mult)
                    mm3 = mmp.tile([128, 1024], BF16, tag="mm", name="mm3")
                    nc.vector.tensor_tensor(out=mm3[:], in0=gt_, in1=h2f, op=ALU.mult)
                    nc.vector.tensor_tensor(out=h2f, in0=mm2[:], in1=mm3[:], op=ALU.add)
                    mm4 = mmp.tile([128, 1024], BF16, tag="mm", name="mm4")
                    nc.vector.tensor_tensor(out=mm4[:], in0=ft, in1=h2f, op=ALU.mult)
                    mm5 = mmp.tile([128, 1024], BF16, tag="mm", name="mm5")
                    nc.vector.tensor_tensor(out=mm5[:], in0=gt_, in1=h3fprev, op=ALU.mult)
                    nc.vector.tensor_tensor(out=h3f, in0=mm4[:], in1=mm5[:], op=ALU.add)
                nc.gpsimd.dma_start(o3f, h3f, accum_op=ALU.add)

        # ---- o[c, b] = sum over (lg, l8, t)
        oacc = [perm.tile([128, BC], F32, tag=f"oacc{cc}", name=f"oacc{cc}")
                for cc in (0, 1)]
        for cc in (0, 1):
            osum = small.tile([128, LH, BC], F32, tag=f"osum{cc}")
            nc.vector.tensor_tensor(out=osum[:], in0=o3[:, 0, cc],
                                    in1=o3[:, 1, cc], op=ALU.add)
            nc.vector.reduce_sum(oacc[cc][:],
                                 osum[:].rearrange("p l b -> p b l"),
                                 axis=mybir.AxisListType.X)

        # ---- head: z^T = fc1_w @ [o; u]^T + b  -> [zc(2 chunks of 128), b]
        f1w = perm.tile([128, 4, D], F32, tag="f1w")
        nc.sync.dma_start(f1w[:], fc1wt.rearrange("(kc k) c -> k kc c", k=128))
        f1b = perm.tile([128, 2], F32, tag="f1b")
        nc.sync.dma_start(f1b[:], fc1b[:])
        catT = [oacc[0], oacc[1], uts[0], uts[1]]
        zT = []
        for cc in (0, 1):
            zp = tps.tile([128, BC], F32, tag="tp")
            for kc in range(4):
                nc.tensor.matmul(
                    zp[:], lhsT=f1w[:, kc, cc * 128:(cc + 1) * 128],
                    rhs=catT[kc][:],
                    start=(kc == 0), stop=(kc == 3))
            zt = small.tile([128, BC], F32, tag=f"zt{cc}")
            nc.scalar.activation(zt[:], zp[:], AF.Identity, bias=f1b[:, cc:cc + 1])
            zT.append(zt)

        # res[b,n] = sum_c w2t[c,(b,n)] * z[c,b]  (mul + ones-matmul partition sum)
        for kc in (0, 1):
            nc.gpsimd.tensor_tensor(
                out=w2t[kc][:].rearrange("p (b n) -> p b n", n=N_TGT),
                in0=w2t[kc][:].rearrange("p (b n) -> p b n", n=N_TGT),
                in1=zT[kc][:, :, None].to_broadcast((128, BC, N_TGT)),
                op=ALU.mult)
        ones = small.tile([128, 1], F32, tag="ones")
        nc.vector.memset(ones[:], 1.0)
        resv = res.rearrange("b n -> (b n)")
        for j in range(4):
            rj = tps.tile([1, 512], F32, tag="tp", name=f"rj{j}")
            for kc in (0, 1):
                nc.tensor.matmul(rj[:], lhsT=ones[:],
                                 rhs=w2t[kc][:, 512 * j:512 * (j + 1)],
                                 start=(kc == 0), stop=(kc == 1))
            rs = small.tile([1, 512], F32, tag="ressb", name=f"rs{j}")
            nc.any.tensor_copy(rs[:], rj[:])
            nc.sync.dma_start(resv[None, 512 * j:512 * (j + 1)], rs[:])


_CACHED_NC = None


def build_nc():
    global _CACHED_NC
    if _CACHED_NC is not None:
        return _CACHED_NC
    nc = bacc.Bacc("TRN2", debug=False, enable_asserts=False)
    with tile.TileContext(nc) as tc:
        _build_kernel(nc, tc)
    nc.compile()
    _CACHED_NC = nc
    return nc


def make_in_maps(seq_var, user_var, item_var, item_emb, user_emb, conv_w,
                 conv_b, fc1_w, fc1_b, W2, b2):
    seq_var = np.asarray(seq_var).astype(np.int32)
    user_var = np.asarray(user_var).astype(np.int32)
    item_var = np.asarray(item_var).astype(np.int32)
    item_emb = np.ascontiguousarray(np.asarray(item_emb, dtype=np.float32))
    user_emb = np.ascontiguousarray(np.asarray(user_emb, dtype=np.float32))
    W2 = np.ascontiguousarray(np.asarray(W2, dtype=np.float32))
    conv_w = np.asarray(conv_w, dtype=np.float32)
    conv_b = np.ascontiguousarray(np.asarray(conv_b, dtype=np.float32))
    fc1_w = np.asarray(fc1_w, dtype=np.float32)
    fc1_b = np.ascontiguousarray(np.asarray(fc1_b, dtype=np.float32))

    import ml_dtypes
    # wt8[tap, p, kt, c] = conv_w[l, m, c, kt*128 + p] * 64  (fp8 e4m3)
    wt8_pack = np.empty((TRI[L], 128, 2, D), ml_dtypes.float8_e4m3fn)
    for l in range(L):
        for m in range(l + 1):
            blk = (conv_w[l, m].T * WSCALE)          # [d, c]
            wt8_pack[TRI[l] + m] = blk.reshape(2, 128, D).transpose(1, 0, 2)
    fc1wt = np.ascontiguousarray(fc1_w.T)
    # convb_pack[c, cc, l] = conv_b[l, cc*128 + c];  fc1b_pack[c, cc] = fc1_b[cc*128+c]
    convb_pack = np.ascontiguousarray(conv_b.reshape(L, 2, 128).transpose(2, 1, 0))
    fc1b_pack = np.ascontiguousarray(fc1_b.reshape(2, 128).T)

    in_maps = []
    for c in range(N_CORES):
        sl = slice(c * BC, (c + 1) * BC)
        in_maps.append({
            "seq8": np.ascontiguousarray(seq_var[sl].reshape(8, 128)),
            "item16": np.ascontiguousarray(item_var[sl].reshape(16, 128)),
            "useri": np.ascontiguousarray(user_var[sl]),
            "item_emb": item_emb,
            "user_emb": user_emb,
            "w2tab": W2,
            "wt8": wt8_pack,
            "convb": convb_pack,
            "fc1wt": fc1wt,
            "fc1b": fc1b_pack,
        })
    return in_maps


def kernel(seq_var, user_var, item_var, item_emb, user_emb, conv_w, conv_b,
           fc1_w, fc1_b, W2, b2, _trace=False):
    from concourse import bass_utils
    nc = build_nc()
    in_maps = make_in_maps(seq_var, user_var, item_var, item_emb, user_emb,
                           conv_w, conv_b, fc1_w, fc1_b, W2, b2)
    r = bass_utils.run_bass_kernel_spmd(
        nc, in_maps, core_ids=list(range(N_CORES)), trace=_trace)
    out = np.concatenate([r.results[c]["res"] for c in range(N_CORES)], axis=0)
    b2 = np.asarray(b2, dtype=np.float32)
    item_var = np.asarray(item_var)
    out = out + b2[item_var][..., 0]
    if _trace:
        return out.astype(np.float32), r
    return out.astype(np.float32)


# revision 18
# speedup vs baseline: 2.0683x; 1.0318x over previous
"""Trainium2 Bass kernel for the QRNN-style recommender model.

Model (per batch row b):
  emb = item_emb[seq]                          # [T=16, D=256]
  conv_out[l,t,c] = sum_{m<=l} emb[t-m] @ W[l,m,c,:] + conv_b[l,c]   (L=16 causal convs)
  f = sigmoid(relu(conv_out))                  # forget gates
  h = fo-pool chain applied 3x over t (QRNN), x0 = emb
  o = sum over (l, t) of h                     # [D]
  z = [o, user_emb[user]] @ fc1_w.T + fc1_b    # [D]
  res[n] = W2[item[n]] . z + b2[item[n]]       # [N_TGT=32]

Sharding: data-parallel over batch B=512 across 8 cores (64 rows each);
all parameters/tables replicated; embedding gathers on-device via
indirect DMA.

Per-core implementation:
  conv: fp8(e4m3) DoubleRow matmuls (contraction 256 in one pass);
        emb scaled x256 and weights x64 on cast, undone by the
        activation scale 1/16384 in the gate pass.
  gates: one sigmoid pass per (l,cc,h) PSUM->SBUF; then
        f = max(sigmoid(z+b), 0.5) == sigmoid(relu(z+b)) via a DVE
        tensor_scalar max; g = 1-f via DVE tensor_scalar mult/add.
  fo-pool: explicit per-timestep recurrence, all 16 L-chains batched
        per op in bf16 (DVE 2x mode); the three chained applications
        advance as a staggered wavefront (h1/h2/h3 per t).
        f*x products and the o += h3 accumulation run on GpSimd.
  L is processed in two groups of 8 so the group-1 conv (74% of MACs)
  overlaps with the group-0 recurrence on DVE.
  head: fc1 via PE, per-row dot with gathered W2 rows via Pool mul +
        ones-vector PE partition-reduction.
"""
import numpy as np

import concourse.bass as bass
import concourse.mybir as mybir
import concourse.tile as tile
from concourse import bacc
from concourse.masks import make_identity

F32 = mybir.dt.float32
BF16 = mybir.dt.bfloat16
F8 = mybir.dt.float8e4
I32 = mybir.dt.int32
AF = mybir.ActivationFunctionType
ALU = mybir.AluOpType
DR = mybir.MatmulPerfMode.DoubleRow

# model dims (hardcoded per problem spec)
N_CORES = 8
B = 512
BC = B // N_CORES          # 64 rows per core
T = 16
L = 16
D = 256
N_TGT = 32
N_ITEMS = 200000
N_USERS = 100000
N_L = 3                    # fo-pool chain depth
TRI = [l * (l + 1) // 2 for l in range(L + 1)]  # block offsets for (l, m<=l)
WSCALE = 64.0              # fp8 weight scale
ESCALE = 256.0             # fp8 emb scale
LG = 2                     # l-groups
LH = L // LG               # 8 l's per group


def _build_kernel(nc, tc):
    seq8 = nc.dram_tensor("seq8", [8, 128], I32, kind="ExternalInput").ap()
    item16 = nc.dram_tensor("item16", [16, 128], I32, kind="ExternalInput").ap()
    useri = nc.dram_tensor("useri", [BC], I32, kind="ExternalInput").ap()
    item_emb = nc.dram_tensor("item_emb", [N_ITEMS, D], F32, kind="ExternalInput").ap()
    user_emb = nc.dram_tensor("user_emb", [N_USERS, D], F32, kind="ExternalInput").ap()
    w2tab = nc.dram_tensor("w2tab", [N_ITEMS, D], F32, kind="ExternalInput").ap()
    wt8 = nc.dram_tensor("wt8", [TRI[L], 128, 2, D], F8, kind="ExternalInput").ap()
    convb = nc.dram_tensor("convb", [128, 2, L], F32, kind="ExternalInput").ap()
    fc1wt = nc.dram_tensor("fc1wt", [2 * D, D], F32, kind="ExternalInput").ap()
    fc1b = nc.dram_tensor("fc1b", [128, 2], F32, kind="ExternalInput").ap()
    res = nc.dram_tensor("res", [BC, N_TGT], F32, kind="ExternalOutput").ap()

    import contextlib
    ctx = contextlib.ExitStack()
    with ctx:
        perm = ctx.enter_context(tc.tile_pool(name="perm", bufs=1))
        idxp = ctx.enter_context(tc.tile_pool(name="idxp", bufs=2))
        gath = ctx.enter_context(tc.tile_pool(name="gath", bufs=2))
        wpool = ctx.enter_context(tc.tile_pool(name="wpool", bufs=8))
        mmp = ctx.enter_context(tc.tile_pool(name="mmp", bufs=6))
        small = ctx.enter_context(tc.tile_pool(name="small", bufs=1))
        cps = ctx.enter_context(tc.tile_pool(name="cps", bufs=3, space="PSUM"))
        tps = ctx.enter_context(tc.tile_pool(name="tps", bufs=2, space="PSUM"))
        stg = ctx.enter_context(tc.tile_pool(name="stg", bufs=4))

        ident = perm.tile([128, 128], F32, tag="ident")
        make_identity(nc, ident)

        # ---- persistent tiles
        # emb8[p, kt, t, b] fp8 (conv rhs); x0[p, t, cc, b] bf16 (chain-1 x)
        emb8 = perm.tile([128, 2, T, BC], F8, tag="emb8")
        x0 = perm.tile([128, T, 2, BC], BF16, tag="x0")
        # gates: f/g[p, t, lg, cc, l8, b] bf16 — lg-major so each (t, lg)
        # step slice is one contiguous 2048-col block (DVE 2x mode)
        f_t = perm.tile([128, T, LG, 2, LH, BC], BF16, tag="f")
        g_t = perm.tile([128, T, LG, 2, LH, BC], BF16, tag="g")
        # o3[p, lg, cc, l8, b] bf16 accumulator
        o3 = perm.tile([128, LG, 2, LH, BC], BF16, tag="o3")
        nc.vector.memset(o3[:], 0.0)
        cb = perm.tile([128, 2, L], F32, tag="cb")
        nc.sync.dma_start(cb[:], convb[:])

        # ---- phase A: gather seq embeddings -> emb8 (fp8, x256) + x0 (bf16)
        tpvs = []
        for c in range(8):
            it = idxp.tile([128, 1], I32, tag="seqidx")
            nc.sync.dma_start(it[:], seq8[c, :, None])
            gt = gath.tile([128, D], F32, tag="embg")
            nc.gpsimd.indirect_dma_start(
                out=gt[:], out_offset=None, in_=item_emb[:],
                in_offset=bass.IndirectOffsetOnAxis(ap=it[:, :1], axis=0))
            for kc in (0, 1):
                tp = tps.tile([128, 128], F32, tag="tp", name=f"tp{c}_{kc}")
                nc.tensor.transpose(tp[:], gt[:, kc * 128:(kc + 1) * 128], ident[:])
                # single fast psum->sbuf copy so the psum buffer frees quickly
                st = stg.tile([128, 128], F32, tag="stg", name=f"st{c}_{kc}")
                nc.scalar.copy(st[:], tp[:])
                # st cols = (b_local 8) x (t 16); emb8 (conv-critical) on Scalar
                stv = st[:].rearrange("p (b t) -> p b t", b=8)
                nc.scalar.mul(
                    emb8[:, kc, :, 8 * c:8 * (c + 1)].rearrange("p t b -> p b t"),
                    stv, ESCALE)
                # x0 (needed later) on idle DVE
                nc.vector.tensor_scalar_mul(
                    x0[:, :, kc, 8 * c:8 * (c + 1)].rearrange("p t b -> p b t"),
                    stv, 1.0)

        w2t = [perm.tile([128, BC * N_TGT], BF16, tag=f"w2t{kc}", name=f"w2t{kc}")
               for kc in (0, 1)]
        uts = []

        def emit_head_inputs():
            # head gathers/transposes/copies; emitted after the lg0 sigmoids so
            # the Scalar copies don't delay the gate pass
            uidx = idxp.tile([BC, 1], I32, tag="uidx")
            nc.sync.dma_start(uidx[:], useri[:, None])
            ug = gath.tile([BC, D], F32, tag="ug")
            nc.gpsimd.indirect_dma_start(
                out=ug[:], out_offset=None, in_=user_emb[:],
                in_offset=bass.IndirectOffsetOnAxis(ap=uidx[:, :1], axis=0))
            for kc in (0, 1):
                tp = tps.tile([128, 128], F32, tag="tp", name=f"tpu{kc}")
                nc.tensor.transpose(tp[:, :BC], ug[:, kc * 128:(kc + 1) * 128],
                                    ident[:BC, :BC])
                ut = small.tile([128, BC], F32, tag=f"ut{kc}", name=f"ut{kc}")
                nc.any.tensor_copy(ut[:], tp[:, :BC])
                uts.append(ut)
            # W2 row gathers -> w2t[kc] = [128, 2048] (c on partitions, (b,n) free)
            for ch in range(16):
                it = idxp.tile([128, 1], I32, tag="itemidx")
                nc.sync.dma_start(it[:], item16[ch, :, None])
                wg = gath.tile([128, D], F32, tag="w2g")
                nc.gpsimd.indirect_dma_start(
                    out=wg[:], out_offset=None, in_=w2tab[:],
                    in_offset=bass.IndirectOffsetOnAxis(ap=it[:, :1], axis=0))
                for kc in (0, 1):
                    tp = tps.tile([128, 128], F32, tag="tp", name=f"tpw{ch}_{kc}")
                    nc.tensor.transpose(tp[:], wg[:, kc * 128:(kc + 1) * 128], ident[:])
                    nc.scalar.copy(w2t[kc][:, 128 * ch:128 * (ch + 1)], tp[:])

        # ---- conv + gates + recurrence per l-group
        olacc = [[], []]
        for lg in range(LG):
            l0 = lg * LH
            # conv: fp8 DoubleRow matmuls; psum tile [128, 1024] spans both
            # b-halves (col = 512*h + 32*t + b), one matmul per (l, m, cc)
            psts = []
            for l in range(l0, l0 + LH):
                w8s = []
                for m in range(l + 1):
                    w8 = wpool.tile([128, 2, D], F8, tag="w8")
                    nc.sync.dma_start(w8[:], wt8[TRI[l] + m])
                    w8s.append(w8)
                pst = [cps.tile([128, 1024], F32, tag="cps", name=f"pst{l}_{i}")
                       for i in (0, 1)]
                for m in range(l + 1):
                    for cc in (0, 1):
                        lhs = w8s[m][:, :, cc * 128:(cc + 1) * 128]
                        for h in (0, 1):
                            rhs = emb8[:, :, 0:T - m, 32 * h:32 * (h + 1)]
                            nc.tensor.matmul(
                                pst[cc][:, 512 * h + 32 * m:512 * (h + 1)],
                                lhsT=lhs, rhs=rhs, perf_mode=DR,
                                start=(m == 0), stop=(m == l))
                psts.append([p_[:].rearrange("p (h t b) -> p t h b", h=2, b=32)
                             for p_ in pst])
            # gates: s = sigmoid(z/16384 + b) into f; f=max(s,.5); g=1-f
            for li, l in enumerate(range(l0, l0 + LH)):
                for cc in (0, 1):
                    nc.scalar.activation(
                        f_t[:, :, lg, cc, li, :].rearrange(
                            "p t (h b) -> p t h b", h=2),
                        psts[li][cc],
                        AF.Sigmoid, bias=cb[:, cc, l:l + 1],
                        scale=1.0 / (WSCALE * ESCALE))
            if lg == 0:
                emit_head_inputs()
            fv = f_t[:, :, lg].rearrange("p t c l b -> p t (c l b)")
            gv = g_t[:, :, lg].rearrange("p t c l b -> p t (c l b)")
            nc.vector.tensor_scalar_max(fv, fv, 0.5)
            nc.vector.tensor_scalar(gv, fv, -1.0, 1.0, op0=ALU.mult, op1=ALU.add)

            # recurrence: h1/h2/h3 wavefront over t, all 8 l's batched; every
            # step slice is a contiguous [128, 1024] block (DVE 2x mode).
            # h3 is double-buffered so the trailing o3 DMA-accum read never
            # stalls the next step's h3 write.
            h1 = perm.tile([128, 2, LH, BC], BF16, tag=f"h1_{lg}", name=f"h1_{lg}")
            h2 = perm.tile([128, 2, LH, BC], BF16, tag=f"h2_{lg}", name=f"h2_{lg}")
            h3p = [perm.tile([128, 2, LH, BC], BF16, tag=f"h3_{lg}_{i}",
                             name=f"h3_{lg}_{i}") for i in (0, 1)]
            h1f, h2f = (h[:].rearrange("p c l b -> p (c l b)") for h in (h1, h2))
            h3pf = [h[:].rearrange("p c l b -> p (c l b)") for h in h3p]
            o3f = o3[:, lg].rearrange("p c l b -> p (c l b)")
            for t in range(T):
                ft = f_t[:, t, lg].rearrange("p c l b -> p (c l b)")
                gt_ = g_t[:, t, lg].rearrange("p c l b -> p (c l b)")
                ft4 = f_t[:, t, lg]
                xb = x0[:, t, :, None, :].to_broadcast((128, 2, LH, BC))
                h3f, h3fprev = h3pf[t % 2], h3pf[1 - t % 2]
                if t == 0:
                    nc.vector.tensor_tensor(out=h1[:], in0=ft4, in1=xb, op=ALU.mult)
                    nc.vector.tensor_tensor(out=h2f, in0=ft, in1=h1f, op=ALU.mult)
                    nc.vector.tensor_tensor(out=h3f, in0=ft, in1=h2f, op=ALU.mult)
                else:
                    m1 = mmp.tile([128, 2, LH, BC], BF16, tag="mm", name="m1")
                    nc.vector.tensor_tensor(out=m1[:], in0=ft4, in1=xb, op=ALU.mult)
                    m1f = m1[:].rearrange("p c l b -> p (c l b)")
                    mm = mmp.tile([128, 1024], BF16, tag="mm", name="mm")
                    nc.vector.tensor_tensor(out=mm[:], in0=gt_, in1=h1f, op=ALU.mult)
                    nc.vector.tensor_tensor(out=h1f, in0=m1f, in1=mm[:], op=ALU.add)
                    mm2 = mmp.tile([128, 1024], BF16, tag="mm", name="mm2")
                    nc.vector.tensor_tensor(out=mm2[:], in0=ft, in1=h1f, op=ALU.mult)
                    mm3 = mmp.tile([128, 1024], BF16, tag="mm", name="mm3")
                    nc.vector.tensor_tensor(out=mm3[:], in0=gt_, in1=h2f, op=ALU.mult)
                    nc.vector.tensor_tensor(out=h2f, in0=mm2[:], in1=mm3[:], op=ALU.add)
                    mm4 = mmp.tile([128, 1024], BF16, tag="mm", name="mm4")
                    nc.vector.tensor_tensor(out=mm4[:], in0=ft, in1=h2f, op=ALU.mult)
                    mm5 = mmp.tile([128, 1024], BF16, tag="mm", name="mm5")
                    nc.vector.tensor_tensor(out=mm5[:], in0=gt_, in1=h3fprev, op=ALU.mult)
                    nc.vector.tensor_tensor(out=h3f, in0=mm4[:], in1=mm5[:], op=ALU.add)
                nc.gpsimd.dma_start(o3f, h3f, accum_op=ALU.add)
            # partial o reduce for this group (runs while the other group works)
            for cc in (0, 1):
                ol = perm.tile([128, BC], F32, tag=f"ol{cc}_{lg}",
                               name=f"ol{cc}_{lg}")
                nc.vector.reduce_sum(ol[:],
                                     o3[:, lg, cc].rearrange("p l b -> p b l"),
                                     axis=mybir.AxisListType.X)
                olacc[cc].append(ol)

        # ---- o[c, b] = sum of the two group partials
        oacc = [perm.tile([128, BC], F32, tag=f"oacc{cc}", name=f"oacc{cc}")
                for cc in (0, 1)]
        for cc in (0, 1):
            nc.vector.tensor_tensor(out=oacc[cc][:], in0=olacc[cc][0][:],
                                    in1=olacc[cc][1][:], op=ALU.add)

        # ---- head: z^T = fc1_w @ [o; u]^T + b  -> [zc(2 chunks of 128), b]
        f1w = perm.tile([128, 4, D], F32, tag="f1w")
        nc.sync.dma_start(f1w[:], fc1wt.rearrange("(kc k) c -> k kc c", k=128))
        f1b = perm.tile([128, 2], F32, tag="f1b")
        nc.sync.dma_start(f1b[:], fc1b[:])
        catT = [oacc[0], oacc[1], uts[0], uts[1]]
        zT = []
        for cc in (0, 1):
            zp = tps.tile([128, BC], F32, tag="tp")
            for kc in range(4):
                nc.tensor.matmul(
                    zp[:], lhsT=f1w[:, kc, cc * 128:(cc + 1) * 128],
                    rhs=catT[kc][:],
                    start=(kc == 0), stop=(kc == 3))
            zt = small.tile([128, BC], BF16, tag=f"zt{cc}")
            nc.scalar.activation(zt[:], zp[:], AF.Identity, bias=f1b[:, cc:cc + 1])
            zT.append(zt)

        # res[b,n] = sum_c w2t[c,(b,n)] * z[c,b]  (mul + ones-matmul partition sum)
        for kc in (0, 1):
            nc.vector.tensor_tensor(
                out=w2t[kc][:].rearrange("p (b n) -> p b n", n=N_TGT),
                in0=w2t[kc][:].rearrange("p (b n) -> p b n", n=N_TGT),
                in1=zT[kc][:, :, None].to_broadcast((128, BC, N_TGT)),
                op=ALU.mult)
        ones = small.tile([128, 1], BF16, tag="ones")
        nc.vector.memset(ones[:], 1.0)
        resv = res.rearrange("b n -> (b n)")
        for j in range(4):
            rj = tps.tile([1, 512], F32, tag="tp", name=f"rj{j}")
            for kc in (0, 1):
                nc.tensor.matmul(rj[:], lhsT=ones[:],
                                 rhs=w2t[kc][:, 512 * j:512 * (j + 1)],
                                 start=(kc == 0), stop=(kc == 1))
            rs = small.tile([1, 512], F32, tag=f"ressb{j}", name=f"rs{j}")
            nc.any.tensor_copy(rs[:], rj[:])
            nc.sync.dma_start(resv[None, 512 * j:512 * (j + 1)], rs[:])


_CACHED_NC = None


def build_nc():
    global _CACHED_NC
    if _CACHED_NC is not None:
        return _CACHED_NC
    nc = bacc.Bacc("TRN2", debug=False, enable_asserts=False)
    with tile.TileContext(nc) as tc:
        _build_kernel(nc, tc)
    nc.compile()
    _CACHED_NC = nc
    return nc


def make_in_maps(seq_var, user_var, item_var, item_emb, user_emb, conv_w,
                 conv_b, fc1_w, fc1_b, W2, b2):
    seq_var = np.asarray(seq_var).astype(np.int32)
    user_var = np.asarray(user_var).astype(np.int32)
    item_var = np.asarray(item_var).astype(np.int32)
    item_emb = np.ascontiguousarray(np.asarray(item_emb, dtype=np.float32))
    user_emb = np.ascontiguousarray(np.asarray(user_emb, dtype=np.float32))
    W2 = np.ascontiguousarray(np.asarray(W2, dtype=np.float32))
    conv_w = np.asarray(conv_w, dtype=np.float32)
    conv_b = np.ascontiguousarray(np.asarray(conv_b, dtype=np.float32))
    fc1_w = np.asarray(fc1_w, dtype=np.float32)
    fc1_b = np.ascontiguousarray(np.asarray(fc1_b, dtype=np.float32))

    import ml_dtypes
    # wt8[tap, p, kt, c] = conv_w[l, m, c, kt*128 + p] * 64  (fp8 e4m3)
    wt8_pack = np.empty((TRI[L], 128, 2, D), ml_dtypes.float8_e4m3fn)
    for l in range(L):
        for m in range(l + 1):
            blk = (conv_w[l, m].T * WSCALE)          # [d, c]
            wt8_pack[TRI[l] + m] = blk.reshape(2, 128, D).transpose(1, 0, 2)
    fc1wt = np.ascontiguousarray(fc1_w.T)
    # convb_pack[c, cc, l] = conv_b[l, cc*128 + c];  fc1b_pack[c, cc] = fc1_b[cc*128+c]
    convb_pack = np.ascontiguousarray(conv_b.reshape(L, 2, 128).transpose(2, 1, 0))
    fc1b_pack = np.ascontiguousarray(fc1_b.reshape(2, 128).T)

    in_maps = []
    for c in range(N_CORES):
        sl = slice(c * BC, (c + 1) * BC)
        in_maps.append({
            "seq8": np.ascontiguousarray(seq_var[sl].reshape(8, 128)),
            "item16": np.ascontiguousarray(item_var[sl].reshape(16, 128)),
            "useri": np.ascontiguousarray(user_var[sl]),
            "item_emb": item_emb,
            "user_emb": user_emb,
            "w2tab": W2,
            "wt8": wt8_pack,
            "convb": convb_pack,
            "fc1wt": fc1wt,
            "fc1b": fc1b_pack,
        })
    return in_maps


def kernel(seq_var, user_var, item_var, item_emb, user_emb, conv_w, conv_b,
           fc1_w, fc1_b, W2, b2, _trace=False):
    from concourse import bass_utils
    nc = build_nc()
    in_maps = make_in_maps(seq_var, user_var, item_var, item_emb, user_emb,
                           conv_w, conv_b, fc1_w, fc1_b, W2, b2)
    r = bass_utils.run_bass_kernel_spmd(
        nc, in_maps, core_ids=list(range(N_CORES)), trace=_trace)
    out = np.concatenate([r.results[c]["res"] for c in range(N_CORES)], axis=0)
    b2 = np.asarray(b2, dtype=np.float32)
    item_var = np.asarray(item_var)
    out = out + b2[item_var][..., 0]
    if _trace:
        return out.astype(np.float32), r
    return out.astype(np.float32)
